# revision 5
# baseline (speedup 1.0000x reference)
"""Multi-head attention (B=4, T=2048, D=1024, H=16) on 8 TRN2 NeuronCores.

Sharding: core c -> (batch b = c//2, head-group g = c%2 of 8 heads).
Each core computes the qkv projection for its batch restricted to its 8
heads, full attention for those heads, and a partial output projection
(ctx_local @ Wout[rows of its heads]).  Host sums the two partials per batch.

v2 schedule, built so the PE engine (the bottleneck at ~315us of inherent
matmul work per core) never stalls:

  - all inputs host-packed so every DMA is per-partition contiguous
    (8-32KB descriptors), issued in priority order (wq, wk, x0, x1, wv,
    x2, x3, wo); x stays resident in SBUF (loaded once, 4MB).
  - "ramp" block: the (pair0, qq0) attention kc-loop interleaves the
    remaining pair-0 qk projection spans and the v projection, so exp
    starts ~12us in instead of ~43us.
  - AV matmuls deferred two k-chunks behind the S matmuls so the ACT
    exp latency is fully hidden by other PE work.
  - the next pair's qk projection is spread evenly over all remaining
    slots of the current pair (48 or 64 slots); the output projection of
    query quarter qq-1 is spread over hc3's qq slots as before.
"""

import numpy as np
import ml_dtypes
from contextlib import ExitStack

import concourse.bass as bass
import concourse.bacc as bacc
import concourse.tile as tile
from concourse import mybir
from concourse.bass_utils import run_bass_kernel_spmd

FP32 = mybir.dt.float32
BF16 = mybir.dt.bfloat16
EXP = mybir.ActivationFunctionType.Exp

D = 1024
T = 2048
HPC = 8          # heads per core
FC = 8           # feature chunks of 128 (projection contraction)
TS = 4           # token spans of 512
KC = 16          # k chunks of 128
QQ = 4           # query quarters of 512


def _norm(nc, rpool, ctx_sb, ctxp, hh, hc, qsl):
    """ctx_sb[hb:hb+64, hc, qsl] = ctxp[0:64] / ctxp[64] (sumexp row)."""
    hb = (hh % 2) * 64
    rtmp = rpool.tile([1, 512], FP32, tag="rtmp")
    nc.vector.tensor_copy(out=rtmp[:], in_=ctxp[64:65, :])
    rt = rpool.tile([1, 512], FP32, tag="rt")
    nc.vector.reciprocal_approx_fast(out=rt[:], in_=rtmp[:])
    rb = rpool.tile([64, 512], FP32, tag="rb")
    nc.gpsimd.partition_broadcast(rb[:], rt[0:1, :], channels=64)
    nc.vector.tensor_mul(ctx_sb[hb:hb + 64, hc, qsl], ctxp[0:64, :], rb[:])


def _attention(nc, ppool, spsum, cpsum, rpool, qT, kT, v_sb, ctx_sb, hc, qq,
               extra=None, post=None):
    """Both heads of pair hc for query quarter qq.

    ``extra`` (called once per k-chunk) interleaves other PE work (the
    pair-0/v projections, the next pair's qk-projection, the output
    projection) into the ACT-bound attention stream.  ``post`` runs before
    the two trailing AV emissions (used by the ramp block for vproj(15))."""
    qsl = slice(qq * 512, (qq + 1) * 512)
    P2 = ppool.tile([128, KC, 2, 512], BF16, tag="P2")
    ctxA = cpsum.tile([65, 512], FP32, tag="ctx")
    ctxB = cpsum.tile([65, 512], FP32, tag="ctx")

    def emit_av(kc):
        for i, ctxp in ((0, ctxA), (1, ctxB)):
            nc.tensor.matmul(
                ctxp[:],
                lhsT=v_sb[:, kc, 2 * hc + i, :],
                rhs=P2[:, kc, i, :],
                start=(kc == 0), stop=(kc == KC - 1))

    for kc in range(KC):
        sps = spsum.tile([128, 2, 512], FP32, tag="S")
        for i in range(2):          # head A on rows 0-63, head B on 64-127
            b0 = i * 64
            nc.tensor.matmul(
                sps[:, i, :],
                lhsT=kT[b0:b0 + 64, hc, kc * 128:(kc + 1) * 128],
                rhs=qT[b0:b0 + 64, hc, qsl],
                start=True, stop=True)
        nc.scalar.activation(
            out=P2[:, kc, :, :], in_=sps[:, :, :], func=EXP, scale=0.125)
        if extra is not None:
            extra(kc)
        # software pipeline: AV deferred two chunks so the in-order PE
        # stream never waits on the ACT exp latency
        if kc >= 2:
            emit_av(kc - 2)
    if post is not None:
        post()
    emit_av(KC - 2)
    emit_av(KC - 1)
    _norm(nc, rpool, ctx_sb, ctxA, 2 * hc, hc, qsl)
    _norm(nc, rpool, ctx_sb, ctxB, 2 * hc + 1, hc, qsl)


def _body(ctx, nc, tc, x_d, wq_d, wk_d, wv_d, wo_d, out_d):
    persist = ctx.enter_context(tc.tile_pool(name="persist", bufs=1))
    x_sb = persist.tile([128, TS, FC, 512], BF16, tag="x")
    qT = persist.tile([128, 4, T], BF16, tag="qT")
    kT = persist.tile([128, 4, T], BF16, tag="kT")
    v_sb = persist.tile([128, KC, HPC, 65], BF16, tag="v")
    ctx_sb = persist.tile([128, 4, T], BF16, tag="ctx")
    wq_sb = persist.tile([128, FC, 512], BF16, tag="wq")
    wk_sb = persist.tile([128, FC, 512], BF16, tag="wk")
    wv_sb = persist.tile([128, FC, 512], BF16, tag="wv")
    wo_sb = persist.tile([128, 4, D], BF16, tag="wo")

    nc.vector.memset(v_sb[:, :, :, 64:65], 1.0)

    # DMA priority order: first qk projection (span 0) needs wq+wk+x0;
    # ramp-block extras then consume x1/wv/x2/x3; wo only at hc3.
    nc.sync.dma_start(out=wq_sb[:], in_=wq_d[:])
    nc.sync.dma_start(out=wk_sb[:], in_=wk_d[:])
    nc.sync.dma_start(out=x_sb[:, 0], in_=x_d[:, 0])
    nc.sync.dma_start(out=x_sb[:, 1], in_=x_d[:, 1])
    nc.sync.dma_start(out=wv_sb[:], in_=wv_d[:])
    nc.sync.dma_start(out=x_sb[:, 2], in_=x_d[:, 2])
    nc.sync.dma_start(out=x_sb[:, 3], in_=x_d[:, 3])
    nc.sync.dma_start(out=wo_sb[:], in_=wo_d[:])

    ps = ctx.enter_context(tc.tile_pool(name="proj", bufs=2, space="PSUM"))

    def make_qk_steps(hc):
        """64 generator steps: one fc-accumulation matmul per step of pair
        hc's qk projection (4 spans x {q,k} x 8 fc), reading resident x."""
        st = {"p": None}

        def step(s):
            unit, fc = divmod(s, FC)
            ts, qk = divmod(unit, 2)
            w_sb, dst = ((wq_sb, qT), (wk_sb, kT))[qk]
            if fc == 0:
                st["p"] = ps.tile([128, 512], FP32, tag="proj", name="qkp")
            nc.tensor.matmul(
                st["p"][:],
                lhsT=w_sb[:, fc, hc * 128:(hc + 1) * 128],
                rhs=x_sb[:, ts, fc, :],
                start=(fc == 0), stop=(fc == FC - 1))
            if fc == FC - 1:
                nc.vector.tensor_copy(
                    out=dst[:, hc, ts * 512:(ts + 1) * 512], in_=st["p"][:])
        return step

    def vproj(kc):
        ts, q4 = divmod(kc, 4)
        psv = ps.tile([128, 512], FP32, tag="proj", name="vp")
        for fc in range(FC):
            nc.tensor.matmul(
                psv[:],
                lhsT=x_sb[:, ts, fc, q4 * 128:(q4 + 1) * 128],
                rhs=wv_sb[:, fc, :],
                start=(fc == 0), stop=(fc == FC - 1))
        nc.vector.tensor_copy(
            out=v_sb[:, kc, :, 0:64],
            in_=psv[:].rearrange("p (h d) -> p h d", h=HPC))

    # pair-0 span-0 q and k projections interleaved per-fc so the first S
    # matmul can start ~0.4us (not ~3.4us) after the x0/wq/wk DMAs land
    qk0 = make_qk_steps(0)
    p_q = ps.tile([128, 512], FP32, tag="proj", name="qkp0q")
    p_k = ps.tile([128, 512], FP32, tag="proj", name="qkp0k")
    for fc in range(FC):
        nc.tensor.matmul(
            p_q[:], lhsT=wq_sb[:, fc, 0:128], rhs=x_sb[:, 0, fc, :],
            start=(fc == 0), stop=(fc == FC - 1))
        nc.tensor.matmul(
            p_k[:], lhsT=wk_sb[:, fc, 0:128], rhs=x_sb[:, 0, fc, :],
            start=(fc == 0), stop=(fc == FC - 1))
    nc.vector.tensor_copy(out=qT[:, 0, 0:512], in_=p_q[:])
    nc.vector.tensor_copy(out=kT[:, 0, 0:512], in_=p_k[:])

    with tc.tile_pool(name="P", bufs=1) as ppool, \
         tc.tile_pool(name="spsum", bufs=2, space="PSUM") as spsum, \
         tc.tile_pool(name="cpsum", bufs=2, space="PSUM") as cpsum, \
         tc.tile_pool(name="rpool", bufs=2) as rpool, \
         tc.tile_pool(name="osb", bufs=4) as osb:

        def ramp_extra(kc):
            # k projections of spans 1-3 (qq0 needs all of kT but only
            # span 0 of qT) over slots 0-11, q of span 1 over slots 12-15;
            # vproj one slot behind the kc index (wv DMA arrives later)
            if kc < 12:
                unit = 2 * (kc // 4) + 3          # k units: ts 1,2,3
                for j in range(2):
                    qk0(unit * FC + 2 * (kc % 4) + j)
            else:
                for j in range(2):                # q unit of span 1
                    qk0(2 * FC + 2 * (kc - 12) + j)
            if kc >= 1:
                vproj(kc - 1)

        def make_op_steps(qq_prev):
            """16 steps emitting the output projection of qq_prev's tokens
            (4 token chunks x 2 column halves x accumulate 4 cc)."""
            st = {"po": None, "ot": None}

            def step(s):
                unit, half = divmod(s, 2)
                tcg = qq_prev * 4 + unit // 2
                j2 = unit % 2
                if half == 0:
                    if j2 == 0:
                        st["ot"] = osb.tile([128, D], FP32, tag="ot", name="ot")
                    st["po"] = ps.tile([128, 512], FP32, tag="proj", name="po")
                    ccs = (0, 1)
                else:
                    ccs = (2, 3)
                for cc in ccs:
                    nc.tensor.matmul(
                        st["po"][:],
                        lhsT=ctx_sb[:, cc, tcg * 128:(tcg + 1) * 128],
                        rhs=wo_sb[:, cc, j2 * 512:(j2 + 1) * 512],
                        start=(cc == 0), stop=(cc == 3))
                if half == 1:
                    nc.vector.tensor_copy(
                        out=st["ot"][:, j2 * 512:(j2 + 1) * 512], in_=st["po"][:])
                    if j2 == 1:
                        nc.sync.dma_start(
                            out=out_d[tcg * 128:(tcg + 1) * 128, :],
                            in_=st["ot"][:])
            return step

        for hc in range(4):
            nxt = make_qk_steps(hc + 1) if hc < 3 else None
            for qq in range(QQ):
                post = None
                if hc == 0 and qq == 0:
                    extra = ramp_extra
                    post = lambda: vproj(15)
                elif hc == 0 and qq >= 1:
                    # 64 steps of pair-1 proj over qq1-3's 48 slots, plus
                    # pair-0's q projection of span qq+1 in the first 4 slots
                    def extra(kc, qq=qq, nxt=nxt):
                        i = (qq - 1) * 16 + kc
                        for s in range((i * 64) // 48, ((i + 1) * 64) // 48):
                            nxt(s)
                        if qq < 3 and kc < 4:
                            unit = 2 * (qq + 1)   # q units of spans 2, 3
                            for j in range(2):
                                qk0(unit * FC + 2 * kc + j)
                elif hc in (1, 2):
                    # 64 steps of the next pair's proj over all 64 slots
                    def extra(kc, qq=qq, nxt=nxt):
                        nxt(qq * 16 + kc)
                elif hc == 3 and qq >= 1:
                    op_step = make_op_steps(qq - 1)

                    def extra(kc, op_step=op_step):
                        if kc < 16:
                            op_step(kc)
                else:
                    extra = None
                _attention(nc, ppool, spsum, cpsum, rpool,
                           qT, kT, v_sb, ctx_sb, hc, qq, extra=extra,
                           post=post)
        # tail: output projection for the last quarter
        op_step = make_op_steps(3)
        for s in range(16):
            op_step(s)


def build():
    nc = bacc.Bacc("TRN2", target_bir_lowering=False, debug=False, num_devices=8)
    x_d = nc.dram_tensor("xt", [128, TS, FC, 512], BF16, kind="ExternalInput").ap()
    wq_d = nc.dram_tensor("wq", [128, FC, 512], BF16, kind="ExternalInput").ap()
    wk_d = nc.dram_tensor("wk", [128, FC, 512], BF16, kind="ExternalInput").ap()
    wv_d = nc.dram_tensor("wv", [128, FC, 512], BF16, kind="ExternalInput").ap()
    wo_d = nc.dram_tensor("wout", [128, 4, D], BF16, kind="ExternalInput").ap()
    out_d = nc.dram_tensor("out", [T, D], FP32, kind="ExternalOutput").ap()
    with tile.TileContext(nc) as tc:
        with ExitStack() as ctx:
            _body(ctx, nc, tc, x_d, wq_d, wk_d, wv_d, wo_d, out_d)
    nc.compile()
    return nc


_nc = None


def _get_nc():
    global _nc
    if _nc is None:
        _nc = build()
    return _nc


def make_in_maps(x, Wqkv, Wout):
    bf = ml_dtypes.bfloat16
    in_maps = []
    for c in range(8):
        b, g = divmod(c, 2)
        cs = slice(g * 512, (g + 1) * 512)
        xt = np.ascontiguousarray(x[b].T)                      # [D, T]
        xp = xt.reshape(FC, 128, TS, 512).transpose(1, 2, 0, 3)  # [p,ts,f,tw]

        def packw(w):   # [D, 512] -> [p, f, c]
            return np.ascontiguousarray(
                w.reshape(FC, 128, 512).transpose(1, 0, 2)).astype(bf)

        wo = Wout[cs, :].reshape(4, 128, D).transpose(1, 0, 2)   # [p, c, d]
        in_maps.append({
            "xt": np.ascontiguousarray(xp).astype(bf),
            "wq": packw(Wqkv[:, 0 * D:1 * D][:, cs]),
            "wk": packw(Wqkv[:, 1 * D:2 * D][:, cs]),
            "wv": packw(Wqkv[:, 2 * D:3 * D][:, cs]),
            "wout": np.ascontiguousarray(wo).astype(bf),
        })
    return in_maps


def kernel(x, Wqkv, Wout, _trace=False):
    nc = _get_nc()
    x = np.asarray(x, dtype=np.float32)
    Wqkv = np.asarray(Wqkv, dtype=np.float32)
    Wout = np.asarray(Wout, dtype=np.float32)
    in_maps = make_in_maps(x, Wqkv, Wout)
    kwargs = {}
    if _trace:
        kwargs["trace"] = True
    res = run_bass_kernel_spmd(nc, in_maps, core_ids=list(range(8)), **kwargs)
    outs = [res.results[c]["out"] for c in range(8)]
    out = np.stack([outs[2 * b] + outs[2 * b + 1] for b in range(4)])
    if _trace:
        kernel.last_result = res
    return out


# revision 7
# speedup vs baseline: 1.0056x; 1.0056x over previous
"""Multi-head attention (B=4, T=2048, D=1024, H=16) on 8 TRN2 NeuronCores.

Sharding: core c -> (batch b = c//2, head-group g = c%2 of 8 heads).
Each core computes the qkv projection for its batch restricted to its 8
heads, full attention for those heads, and a partial output projection
(ctx_local @ Wout[rows of its heads]).  Host sums the two partials per batch.

v2 schedule, built so the PE engine (the bottleneck at ~315us of inherent
matmul work per core) never stalls:

  - all inputs host-packed so every DMA is per-partition contiguous
    (8-32KB descriptors), issued in priority order (wq, wk, x0, x1, wv,
    x2, x3, wo); x stays resident in SBUF (loaded once, 4MB).
  - "ramp" block: the (pair0, qq0) attention kc-loop interleaves the
    remaining pair-0 qk projection spans and the v projection, so exp
    starts ~12us in instead of ~43us.
  - AV matmuls deferred two k-chunks behind the S matmuls so the ACT
    exp latency is fully hidden by other PE work.
  - the next pair's qk projection is spread evenly over all remaining
    slots of the current pair (48 or 64 slots); the output projection of
    query quarter qq-1 is spread over hc3's qq slots as before.
"""

import numpy as np
import ml_dtypes
from contextlib import ExitStack

import concourse.bass as bass
import concourse.bacc as bacc
import concourse.tile as tile
from concourse import mybir
from concourse.bass_utils import run_bass_kernel_spmd

FP32 = mybir.dt.float32
BF16 = mybir.dt.bfloat16
EXP = mybir.ActivationFunctionType.Exp

D = 1024
T = 2048
HPC = 8          # heads per core
FC = 8           # feature chunks of 128 (projection contraction)
TS = 4           # token spans of 512
KC = 16          # k chunks of 128
QQ = 4           # query quarters of 512


def _norm(nc, rpool, ctx_sb, ctxp, hh, hc, qsl):
    """ctx_sb[hb:hb+64, hc, qsl] = ctxp[0:64] / ctxp[64] (sumexp row)."""
    hb = (hh % 2) * 64
    rtmp = rpool.tile([1, 512], FP32, tag="rtmp")
    nc.vector.tensor_copy(out=rtmp[:], in_=ctxp[64:65, :])
    rt = rpool.tile([1, 512], FP32, tag="rt")
    nc.vector.reciprocal_approx_fast(out=rt[:], in_=rtmp[:])
    rb = rpool.tile([64, 512], FP32, tag="rb")
    nc.gpsimd.partition_broadcast(rb[:], rt[0:1, :], channels=64)
    nc.vector.tensor_mul(ctx_sb[hb:hb + 64, hc, qsl], ctxp[0:64, :], rb[:])


def _attention(nc, ppool, spsum, cpsum, rpool, qT, kT, v_sb, ctx_sb, hc, qq,
               extra=None, post=None):
    """Both heads of pair hc for query quarter qq.

    ``extra`` (called once per k-chunk) interleaves other PE work (the
    pair-0/v projections, the next pair's qk-projection, the output
    projection) into the ACT-bound attention stream.  ``post`` runs before
    the two trailing AV emissions (used by the ramp block for vproj(15))."""
    qsl = slice(qq * 512, (qq + 1) * 512)
    P2 = ppool.tile([128, KC, 2, 512], BF16, tag="P2")
    ctxA = cpsum.tile([65, 512], FP32, tag="ctx")
    ctxB = cpsum.tile([65, 512], FP32, tag="ctx")

    def emit_av(kc):
        for i, ctxp in ((0, ctxA), (1, ctxB)):
            nc.tensor.matmul(
                ctxp[:],
                lhsT=v_sb[:, kc, 2 * hc + i, :],
                rhs=P2[:, kc, i, :],
                start=(kc == 0), stop=(kc == KC - 1))

    for kc in range(KC):
        sps = spsum.tile([128, 2, 512], FP32, tag="S")
        for i in range(2):          # head A on rows 0-63, head B on 64-127
            b0 = i * 64
            nc.tensor.matmul(
                sps[:, i, :],
                lhsT=kT[b0:b0 + 64, hc, kc * 128:(kc + 1) * 128],
                rhs=qT[b0:b0 + 64, hc, qsl],
                start=True, stop=True)
        nc.scalar.activation(
            out=P2[:, kc, :, :], in_=sps[:, :, :], func=EXP, scale=0.125)
        if extra is not None:
            extra(kc)
        # software pipeline: AV deferred two chunks so the in-order PE
        # stream never waits on the ACT exp latency
        if kc >= 2:
            emit_av(kc - 2)
    if post is not None:
        post()
    emit_av(KC - 2)
    emit_av(KC - 1)
    _norm(nc, rpool, ctx_sb, ctxA, 2 * hc, hc, qsl)
    _norm(nc, rpool, ctx_sb, ctxB, 2 * hc + 1, hc, qsl)


def _body(ctx, nc, tc, x_d, wq_d, wk_d, wv_d, wo_d, out_d):
    persist = ctx.enter_context(tc.tile_pool(name="persist", bufs=1))
    x_sb = persist.tile([128, TS, FC, 512], BF16, tag="x")
    qT = persist.tile([128, 4, T], BF16, tag="qT")
    kT = persist.tile([128, 4, T], BF16, tag="kT")
    v_sb = persist.tile([128, KC, HPC, 65], BF16, tag="v")
    ctx_sb = persist.tile([128, 4, T], BF16, tag="ctx")
    wq_sb = persist.tile([128, FC, 512], BF16, tag="wq")
    wk_sb = persist.tile([128, FC, 512], BF16, tag="wk")
    wv_sb = persist.tile([128, FC, 512], BF16, tag="wv")
    wo_sb = persist.tile([128, 4, D], BF16, tag="wo")

    nc.vector.memset(v_sb[:, :, :, 64:65], 1.0)

    # DMA priority order: first qk projection (span 0) needs wq+wk+x0;
    # ramp-block extras then consume x1/wv/x2/x3; wo only at hc3.
    nc.sync.dma_start(out=x_sb[:, 0], in_=x_d[:, 0])
    nc.sync.dma_start(out=wq_sb[:], in_=wq_d[:])
    nc.sync.dma_start(out=wk_sb[:], in_=wk_d[:])
    nc.sync.dma_start(out=x_sb[:, 1], in_=x_d[:, 1])
    nc.sync.dma_start(out=wv_sb[:], in_=wv_d[:])
    nc.sync.dma_start(out=x_sb[:, 2], in_=x_d[:, 2])
    nc.sync.dma_start(out=x_sb[:, 3], in_=x_d[:, 3])
    nc.sync.dma_start(out=wo_sb[:], in_=wo_d[:])

    ps = ctx.enter_context(tc.tile_pool(name="proj", bufs=2, space="PSUM"))

    def make_qk_steps(hc):
        """64 generator steps: one fc-accumulation matmul per step of pair
        hc's qk projection (4 spans x {q,k} x 8 fc), reading resident x."""
        st = {"p": None}

        def step(s):
            unit, fc = divmod(s, FC)
            ts, qk = divmod(unit, 2)
            w_sb, dst = ((wq_sb, qT), (wk_sb, kT))[qk]
            if fc == 0:
                st["p"] = ps.tile([128, 512], FP32, tag="proj", name="qkp")
            nc.tensor.matmul(
                st["p"][:],
                lhsT=w_sb[:, fc, hc * 128:(hc + 1) * 128],
                rhs=x_sb[:, ts, fc, :],
                start=(fc == 0), stop=(fc == FC - 1))
            if fc == FC - 1:
                nc.vector.tensor_copy(
                    out=dst[:, hc, ts * 512:(ts + 1) * 512], in_=st["p"][:])
        return step

    def vproj(kc):
        ts, q4 = divmod(kc, 4)
        psv = ps.tile([128, 512], FP32, tag="proj", name="vp")
        for fc in range(FC):
            nc.tensor.matmul(
                psv[:],
                lhsT=x_sb[:, ts, fc, q4 * 128:(q4 + 1) * 128],
                rhs=wv_sb[:, fc, :],
                start=(fc == 0), stop=(fc == FC - 1))
        nc.vector.tensor_copy(
            out=v_sb[:, kc, :, 0:64],
            in_=psv[:].rearrange("p (h d) -> p h d", h=HPC))

    # pair-0 span-0 q and k projections interleaved per-fc so the first S
    # matmul can start ~0.4us (not ~3.4us) after the x0/wq/wk DMAs land
    qk0 = make_qk_steps(0)
    p_q = ps.tile([128, 512], FP32, tag="proj", name="qkp0q")
    p_k = ps.tile([128, 512], FP32, tag="proj", name="qkp0k")
    for fc in range(FC):
        nc.tensor.matmul(
            p_q[:], lhsT=wq_sb[:, fc, 0:128], rhs=x_sb[:, 0, fc, :],
            start=(fc == 0), stop=(fc == FC - 1))
        nc.tensor.matmul(
            p_k[:], lhsT=wk_sb[:, fc, 0:128], rhs=x_sb[:, 0, fc, :],
            start=(fc == 0), stop=(fc == FC - 1))
    nc.vector.tensor_copy(out=qT[:, 0, 0:512], in_=p_q[:])
    nc.vector.tensor_copy(out=kT[:, 0, 0:512], in_=p_k[:])

    with tc.tile_pool(name="P", bufs=1) as ppool, \
         tc.tile_pool(name="spsum", bufs=2, space="PSUM") as spsum, \
         tc.tile_pool(name="cpsum", bufs=2, space="PSUM") as cpsum, \
         tc.tile_pool(name="rpool", bufs=2) as rpool, \
         tc.tile_pool(name="osb", bufs=4) as osb:

        def ramp_extra(kc):
            # k projections of spans 1-3 (qq0 needs all of kT but only
            # span 0 of qT) over slots 0-11, q of span 1 over slots 12-15;
            # vproj one slot behind the kc index (wv DMA arrives later)
            if kc < 12:
                unit = 2 * (kc // 4) + 3          # k units: ts 1,2,3
                for j in range(2):
                    qk0(unit * FC + 2 * (kc % 4) + j)
            else:
                for j in range(2):                # q unit of span 1
                    qk0(2 * FC + 2 * (kc - 12) + j)
            if kc >= 1:
                vproj(kc - 1)

        def make_op_steps(qq_prev):
            """16 steps emitting the output projection of qq_prev's tokens
            (4 token chunks x 2 column halves x accumulate 4 cc)."""
            st = {"po": None, "ot": None}

            def step(s):
                unit, half = divmod(s, 2)
                tcg = qq_prev * 4 + unit // 2
                j2 = unit % 2
                if half == 0:
                    if j2 == 0:
                        st["ot"] = osb.tile([128, D], FP32, tag="ot", name="ot")
                    st["po"] = ps.tile([128, 512], FP32, tag="proj", name="po")
                    ccs = (0, 1)
                else:
                    ccs = (2, 3)
                for cc in ccs:
                    nc.tensor.matmul(
                        st["po"][:],
                        lhsT=ctx_sb[:, cc, tcg * 128:(tcg + 1) * 128],
                        rhs=wo_sb[:, cc, j2 * 512:(j2 + 1) * 512],
                        start=(cc == 0), stop=(cc == 3))
                if half == 1:
                    nc.vector.tensor_copy(
                        out=st["ot"][:, j2 * 512:(j2 + 1) * 512], in_=st["po"][:])
                    if j2 == 1:
                        nc.sync.dma_start(
                            out=out_d[tcg * 128:(tcg + 1) * 128, :],
                            in_=st["ot"][:])
            return step

        for hc in range(4):
            nxt = make_qk_steps(hc + 1) if hc < 3 else None
            for qq in range(QQ):
                post = None
                if hc == 0 and qq == 0:
                    extra = ramp_extra
                    post = lambda: vproj(15)
                elif hc == 0 and qq >= 1:
                    # 64 steps of pair-1 proj over qq1-3's 48 slots, plus
                    # pair-0's q projection of span qq+1 in the first 4 slots
                    def extra(kc, qq=qq, nxt=nxt):
                        i = (qq - 1) * 16 + kc
                        for s in range((i * 64) // 48, ((i + 1) * 64) // 48):
                            nxt(s)
                        if qq < 3 and kc < 4:
                            unit = 2 * (qq + 1)   # q units of spans 2, 3
                            for j in range(2):
                                qk0(unit * FC + 2 * kc + j)
                elif hc in (1, 2):
                    # 64 steps of the next pair's proj over all 64 slots
                    def extra(kc, qq=qq, nxt=nxt):
                        nxt(qq * 16 + kc)
                elif hc == 3 and qq >= 1:
                    op_step = make_op_steps(qq - 1)

                    # start two slots in: the first op matmul reads ctx_sb
                    # written by the previous block's norm chain, which has
                    # ~2.3us of DVE+gpsimd latency past the last AV
                    def extra(kc, op_step=op_step):
                        if kc >= 2:
                            op_step(kc - 2)
                    post = lambda op_step=op_step: (op_step(14), op_step(15))
                else:
                    extra = None
                _attention(nc, ppool, spsum, cpsum, rpool,
                           qT, kT, v_sb, ctx_sb, hc, qq, extra=extra,
                           post=post)
        # tail: output projection for qq3.  cc0-2 depend only on earlier
        # pairs' ctx, so pre-emit two units' worth to overlap the final
        # norm chain; cc3 of each unit waits on that norm.
        pos = {}

        def emit_cc012(u):
            tcg, j2 = 12 + u // 2, u % 2
            po = ps.tile([128, 512], FP32, tag="proj", name="po")
            for cc in range(3):
                nc.tensor.matmul(
                    po[:],
                    lhsT=ctx_sb[:, cc, tcg * 128:(tcg + 1) * 128],
                    rhs=wo_sb[:, cc, j2 * 512:(j2 + 1) * 512],
                    start=(cc == 0), stop=False)
            pos[u] = po

        emit_cc012(0)
        emit_cc012(1)
        ot = None
        for u in range(8):
            tcg, j2 = 12 + u // 2, u % 2
            if u >= 2:
                emit_cc012(u)
            po = pos.pop(u)
            nc.tensor.matmul(
                po[:],
                lhsT=ctx_sb[:, 3, tcg * 128:(tcg + 1) * 128],
                rhs=wo_sb[:, 3, j2 * 512:(j2 + 1) * 512],
                start=False, stop=True)
            if j2 == 0:
                ot = osb.tile([128, D], FP32, tag="ot", name="ot")
            nc.vector.tensor_copy(out=ot[:, j2 * 512:(j2 + 1) * 512], in_=po[:])
            if j2 == 1:
                nc.sync.dma_start(
                    out=out_d[tcg * 128:(tcg + 1) * 128, :], in_=ot[:])


def build():
    nc = bacc.Bacc("TRN2", target_bir_lowering=False, debug=False, num_devices=8)
    x_d = nc.dram_tensor("xt", [128, TS, FC, 512], BF16, kind="ExternalInput").ap()
    wq_d = nc.dram_tensor("wq", [128, FC, 512], BF16, kind="ExternalInput").ap()
    wk_d = nc.dram_tensor("wk", [128, FC, 512], BF16, kind="ExternalInput").ap()
    wv_d = nc.dram_tensor("wv", [128, FC, 512], BF16, kind="ExternalInput").ap()
    wo_d = nc.dram_tensor("wout", [128, 4, D], BF16, kind="ExternalInput").ap()
    out_d = nc.dram_tensor("out", [T, D], FP32, kind="ExternalOutput").ap()
    with tile.TileContext(nc) as tc:
        with ExitStack() as ctx:
            _body(ctx, nc, tc, x_d, wq_d, wk_d, wv_d, wo_d, out_d)
    nc.compile()
    return nc


_nc = None


def _get_nc():
    global _nc
    if _nc is None:
        _nc = build()
    return _nc


def make_in_maps(x, Wqkv, Wout):
    bf = ml_dtypes.bfloat16
    in_maps = []
    for c in range(8):
        b, g = divmod(c, 2)
        cs = slice(g * 512, (g + 1) * 512)
        xt = np.ascontiguousarray(x[b].T)                      # [D, T]
        xp = xt.reshape(FC, 128, TS, 512).transpose(1, 2, 0, 3)  # [p,ts,f,tw]

        def packw(w):   # [D, 512] -> [p, f, c]
            return np.ascontiguousarray(
                w.reshape(FC, 128, 512).transpose(1, 0, 2)).astype(bf)

        wo = Wout[cs, :].reshape(4, 128, D).transpose(1, 0, 2)   # [p, c, d]
        in_maps.append({
            "xt": np.ascontiguousarray(xp).astype(bf),
            "wq": packw(Wqkv[:, 0 * D:1 * D][:, cs]),
            "wk": packw(Wqkv[:, 1 * D:2 * D][:, cs]),
            "wv": packw(Wqkv[:, 2 * D:3 * D][:, cs]),
            "wout": np.ascontiguousarray(wo).astype(bf),
        })
    return in_maps


def kernel(x, Wqkv, Wout, _trace=False):
    nc = _get_nc()
    x = np.asarray(x, dtype=np.float32)
    Wqkv = np.asarray(Wqkv, dtype=np.float32)
    Wout = np.asarray(Wout, dtype=np.float32)
    in_maps = make_in_maps(x, Wqkv, Wout)
    kwargs = {}
    if _trace:
        kwargs["trace"] = True
    res = run_bass_kernel_spmd(nc, in_maps, core_ids=list(range(8)), **kwargs)
    outs = [res.results[c]["out"] for c in range(8)]
    out = np.stack([outs[2 * b] + outs[2 * b + 1] for b in range(4)])
    if _trace:
        kernel.last_result = res
    return out


# revision 11
# speedup vs baseline: 1.0239x; 1.0182x over previous
"""Multi-head attention (B=4, T=2048, D=1024, H=16) on 8 TRN2 NeuronCores.

Sharding: core c -> (batch b = c//2, head-group g = c%2 of 8 heads).
Each core computes the qkv projection for its batch restricted to its 8
heads, full attention for those heads, and a partial output projection
(ctx_local @ Wout[rows of its heads]).  Host sums the two partials per batch.

v2 schedule, built so the PE engine (the bottleneck at ~315us of inherent
matmul work per core) never stalls:

  - all inputs host-packed so every DMA is per-partition contiguous
    (8-32KB descriptors), issued in priority order (wq, wk, x0, x1, wv,
    x2, x3, wo); x stays resident in SBUF (loaded once, 4MB).
  - "ramp" block: the (pair0, qq0) attention kc-loop interleaves the
    remaining pair-0 qk projection spans and the v projection, so exp
    starts ~12us in instead of ~43us.
  - AV matmuls deferred two k-chunks behind the S matmuls so the ACT
    exp latency is fully hidden by other PE work.
  - the next pair's qk projection is spread evenly over all remaining
    slots of the current pair (48 or 64 slots); the output projection of
    query quarter qq-1 is spread over hc3's qq slots as before.
"""

import numpy as np
import ml_dtypes
from contextlib import ExitStack

import concourse.bass as bass
import concourse.bacc as bacc
import concourse.tile as tile
from concourse import mybir
from concourse.bass_utils import run_bass_kernel_spmd

FP32 = mybir.dt.float32
BF16 = mybir.dt.bfloat16
EXP = mybir.ActivationFunctionType.Exp

D = 1024
T = 2048
HPC = 8          # heads per core
FC = 8           # feature chunks of 128 (projection contraction)
TS = 4           # token spans of 512
KC = 16          # k chunks of 128
QQ = 4           # query quarters of 512


def _norm(nc, rpool, ctx_sb, ctxp, hh, hc, qsl):
    """ctx_sb[hb:hb+64, hc, qsl] = ctxp[0:64] / ctxp[64] (sumexp row)."""
    hb = (hh % 2) * 64
    rtmp = rpool.tile([1, 512], FP32, tag="rtmp")
    nc.vector.tensor_copy(out=rtmp[:], in_=ctxp[64:65, :])
    rt = rpool.tile([1, 512], FP32, tag="rt")
    nc.vector.reciprocal_approx_fast(out=rt[:], in_=rtmp[:])
    rb = rpool.tile([64, 512], FP32, tag="rb")
    nc.gpsimd.partition_broadcast(rb[:], rt[0:1, :], channels=64)
    nc.vector.tensor_mul(ctx_sb[hb:hb + 64, hc, qsl], ctxp[0:64, :], rb[:])


def _attention(nc, ppool, spsum, cpsum, rpool, qT, kT, v_sb, ctx_sb, hc, qq,
               extra=None, post=None):
    """Both heads of pair hc for query quarter qq.

    ``extra`` (called once per k-chunk) interleaves other PE work (the
    pair-0/v projections, the next pair's qk-projection, the output
    projection) into the ACT-bound attention stream.  ``post`` runs before
    the two trailing AV emissions (used by the ramp block for vproj(15))."""
    qsl = slice(qq * 512, (qq + 1) * 512)
    P2 = ppool.tile([128, KC, 2, 512], BF16, tag="P2")
    ctxA = cpsum.tile([65, 512], FP32, tag="ctx")
    ctxB = cpsum.tile([65, 512], FP32, tag="ctx")

    def emit_av(kc):
        for i, ctxp in ((0, ctxA), (1, ctxB)):
            nc.tensor.matmul(
                ctxp[:],
                lhsT=v_sb[:, kc, 2 * hc + i, :],
                rhs=P2[:, kc, i, :],
                start=(kc == 0), stop=(kc == KC - 1))

    for kc in range(KC):
        sps = spsum.tile([128, 2, 512], FP32, tag="S")
        for i in range(2):          # head A on rows 0-63, head B on 64-127
            b0 = i * 64
            nc.tensor.matmul(
                sps[:, i, :],
                lhsT=kT[b0:b0 + 64, hc, kc * 128:(kc + 1) * 128],
                rhs=qT[b0:b0 + 64, hc, qsl],
                start=True, stop=True)
        nc.scalar.activation(
            out=P2[:, kc, :, :], in_=sps[:, :, :], func=EXP, scale=0.125)
        if extra is not None:
            extra(kc)
        # software pipeline: AV deferred three chunks, both to hide the ACT
        # exp latency and so the next block's first AV (which reuses this
        # block's cpsum buffer) lands after the ~4us norm chain completes
        if kc >= 3:
            emit_av(kc - 3)
    if post is not None:
        post()
    emit_av(KC - 3)
    emit_av(KC - 2)
    emit_av(KC - 1)
    _norm(nc, rpool, ctx_sb, ctxA, 2 * hc, hc, qsl)
    _norm(nc, rpool, ctx_sb, ctxB, 2 * hc + 1, hc, qsl)


def _body(ctx, nc, tc, x_d, wq_d, wk_d, wv_d, wo_d, out_d):
    persist = ctx.enter_context(tc.tile_pool(name="persist", bufs=1))
    x_sb = persist.tile([128, TS, FC, 512], BF16, tag="x")
    qT = persist.tile([128, 4, T], BF16, tag="qT")
    kT = persist.tile([128, 4, T], BF16, tag="kT")
    v_sb = persist.tile([128, KC, HPC, 65], BF16, tag="v")
    ctx_sb = persist.tile([128, 4, T], BF16, tag="ctx")
    wq_sb = persist.tile([128, FC, 512], BF16, tag="wq")
    wk_sb = persist.tile([128, FC, 512], BF16, tag="wk")
    wv_sb = persist.tile([128, FC, 512], BF16, tag="wv")
    wo_sb = persist.tile([128, 4, D], BF16, tag="wo")

    nc.vector.memset(v_sb[:, :, :, 64:65], 1.0)

    # DMA priority order: first qk projection (span 0) needs wq+wk+x0;
    # ramp-block extras then consume x1/wv/x2/x3; wo only at hc3.
    nc.sync.dma_start(out=x_sb[:, 0], in_=x_d[:, 0])
    nc.sync.dma_start(out=wq_sb[:], in_=wq_d[:])
    nc.sync.dma_start(out=wk_sb[:], in_=wk_d[:])
    nc.sync.dma_start(out=x_sb[:, 1], in_=x_d[:, 1])
    nc.sync.dma_start(out=wv_sb[:], in_=wv_d[:])
    nc.sync.dma_start(out=x_sb[:, 2], in_=x_d[:, 2])
    nc.sync.dma_start(out=x_sb[:, 3], in_=x_d[:, 3])
    nc.sync.dma_start(out=wo_sb[:], in_=wo_d[:])

    ps = ctx.enter_context(tc.tile_pool(name="proj", bufs=2, space="PSUM"))

    def make_qk_steps(hc):
        """64 generator steps: one fc-accumulation matmul per step of pair
        hc's qk projection (4 spans x {q,k} x 8 fc), reading resident x."""
        st = {"p": None}

        def step(s):
            unit, fc = divmod(s, FC)
            ts, qk = divmod(unit, 2)
            w_sb, dst = ((wq_sb, qT), (wk_sb, kT))[qk]
            if fc == 0:
                st["p"] = ps.tile([128, 512], FP32, tag="proj", name="qkp")
            nc.tensor.matmul(
                st["p"][:],
                lhsT=w_sb[:, fc, hc * 128:(hc + 1) * 128],
                rhs=x_sb[:, ts, fc, :],
                start=(fc == 0), stop=(fc == FC - 1))
            if fc == FC - 1:
                nc.vector.tensor_copy(
                    out=dst[:, hc, ts * 512:(ts + 1) * 512], in_=st["p"][:])
        return step

    def vproj(kc):
        ts, q4 = divmod(kc, 4)
        psv = ps.tile([128, 512], FP32, tag="proj", name="vp")
        for fc in range(FC):
            nc.tensor.matmul(
                psv[:],
                lhsT=x_sb[:, ts, fc, q4 * 128:(q4 + 1) * 128],
                rhs=wv_sb[:, fc, :],
                start=(fc == 0), stop=(fc == FC - 1))
        nc.vector.tensor_copy(
            out=v_sb[:, kc, :, 0:64],
            in_=psv[:].rearrange("p (h d) -> p h d", h=HPC))

    # pair-0 span-0 q and k projections interleaved per-fc so the first S
    # matmul can start ~0.4us (not ~3.4us) after the x0/wq/wk DMAs land
    qk0 = make_qk_steps(0)
    p_q = ps.tile([128, 512], FP32, tag="proj", name="qkp0q")
    p_k = ps.tile([128, 512], FP32, tag="proj", name="qkp0k")
    for fc in range(FC):
        nc.tensor.matmul(
            p_q[:], lhsT=wq_sb[:, fc, 0:128], rhs=x_sb[:, 0, fc, :],
            start=(fc == 0), stop=(fc == FC - 1))
        nc.tensor.matmul(
            p_k[:], lhsT=wk_sb[:, fc, 0:128], rhs=x_sb[:, 0, fc, :],
            start=(fc == 0), stop=(fc == FC - 1))
    nc.vector.tensor_copy(out=qT[:, 0, 0:512], in_=p_q[:])
    nc.vector.tensor_copy(out=kT[:, 0, 0:512], in_=p_k[:])

    with tc.tile_pool(name="P", bufs=1) as ppool, \
         tc.tile_pool(name="spsum", bufs=2, space="PSUM") as spsum, \
         tc.tile_pool(name="cpsum", bufs=2, space="PSUM") as cpsum, \
         tc.tile_pool(name="rpool", bufs=2) as rpool, \
         tc.tile_pool(name="osb", bufs=4) as osb:

        def ramp_extra(kc):
            # k projections of spans 1-3 (qq0 needs all of kT but only
            # span 0 of qT) over slots 0-11, q of span 1 over slots 12-15;
            # vproj one slot behind the kc index (wv DMA arrives later)
            if kc < 12:
                unit = 2 * (kc // 4) + 3          # k units: ts 1,2,3
                for j in range(2):
                    qk0(unit * FC + 2 * (kc % 4) + j)
            else:
                for j in range(2):                # q unit of span 1
                    qk0(2 * FC + 2 * (kc - 12) + j)
            if kc >= 1:
                vproj(kc - 1)

        def make_op_steps(qq_prev):
            """16 steps emitting the output projection of qq_prev's tokens
            (4 token chunks x 2 column halves x accumulate 4 cc)."""
            st = {"po": None, "ot": None}

            def step(s):
                unit, half = divmod(s, 2)
                tcg = qq_prev * 4 + unit // 2
                j2 = unit % 2
                if half == 0:
                    if j2 == 0:
                        st["ot"] = osb.tile([128, D], FP32, tag="ot", name="ot")
                    st["po"] = ps.tile([128, 512], FP32, tag="proj", name="po")
                    ccs = (0, 1)
                else:
                    ccs = (2, 3)
                for cc in ccs:
                    nc.tensor.matmul(
                        st["po"][:],
                        lhsT=ctx_sb[:, cc, tcg * 128:(tcg + 1) * 128],
                        rhs=wo_sb[:, cc, j2 * 512:(j2 + 1) * 512],
                        start=(cc == 0), stop=(cc == 3))
                if half == 1:
                    nc.vector.tensor_copy(
                        out=st["ot"][:, j2 * 512:(j2 + 1) * 512], in_=st["po"][:])
                    if j2 == 1:
                        nc.sync.dma_start(
                            out=out_d[tcg * 128:(tcg + 1) * 128, :],
                            in_=st["ot"][:])
            return step

        for hc in range(4):
            nxt = make_qk_steps(hc + 1) if hc < 3 else None
            for qq in range(QQ):
                post = None
                if hc == 0 and qq == 0:
                    extra = ramp_extra
                    post = lambda: vproj(15)
                elif hc == 0 and qq >= 1:
                    # 64 steps of pair-1 proj over qq1-3's 48 slots, plus
                    # pair-0's q projection of span qq+1 in the first 4 slots
                    def extra(kc, qq=qq, nxt=nxt):
                        i = (qq - 1) * 16 + kc
                        for s in range((i * 64) // 48, ((i + 1) * 64) // 48):
                            nxt(s)
                        if qq < 3 and kc < 4:
                            unit = 2 * (qq + 1)   # q units of spans 2, 3
                            for j in range(2):
                                qk0(unit * FC + 2 * kc + j)
                elif hc in (1, 2):
                    # 64 steps of the next pair's proj over all 64 slots
                    def extra(kc, qq=qq, nxt=nxt):
                        nxt(qq * 16 + kc)
                elif hc == 3 and qq >= 1:
                    op_step = make_op_steps(qq - 1)

                    # start two slots in: the first op matmul reads ctx_sb
                    # written by the previous block's norm chain, which has
                    # ~2.3us of DVE+gpsimd latency past the last AV
                    def extra(kc, op_step=op_step):
                        if kc >= 2:
                            op_step(kc - 2)
                    post = lambda op_step=op_step: (op_step(14), op_step(15))
                else:
                    extra = None
                _attention(nc, ppool, spsum, cpsum, rpool,
                           qT, kT, v_sb, ctx_sb, hc, qq, extra=extra,
                           post=post)
        # tail: output projection for qq3.  cc0-2 depend only on earlier
        # pairs' ctx, so pre-emit two units' worth to overlap the final
        # norm chain; cc3 of each unit waits on that norm.
        pos = {}

        def emit_cc012(u):
            tcg, j2 = 12 + u // 2, u % 2
            po = ps.tile([128, 512], FP32, tag="proj", name="po")
            for cc in range(3):
                nc.tensor.matmul(
                    po[:],
                    lhsT=ctx_sb[:, cc, tcg * 128:(tcg + 1) * 128],
                    rhs=wo_sb[:, cc, j2 * 512:(j2 + 1) * 512],
                    start=(cc == 0), stop=False)
            pos[u] = po

        emit_cc012(0)
        emit_cc012(1)
        ot = None
        for u in range(8):
            tcg, j2 = 12 + u // 2, u % 2
            if u >= 2:
                emit_cc012(u)
            po = pos.pop(u)
            nc.tensor.matmul(
                po[:],
                lhsT=ctx_sb[:, 3, tcg * 128:(tcg + 1) * 128],
                rhs=wo_sb[:, 3, j2 * 512:(j2 + 1) * 512],
                start=False, stop=True)
            if j2 == 0:
                ot = osb.tile([128, D], FP32, tag="ot", name="ot")
            nc.vector.tensor_copy(out=ot[:, j2 * 512:(j2 + 1) * 512], in_=po[:])
            if j2 == 1:
                nc.sync.dma_start(
                    out=out_d[tcg * 128:(tcg + 1) * 128, :], in_=ot[:])


def build():
    nc = bacc.Bacc("TRN2", target_bir_lowering=False, debug=False, num_devices=8)
    x_d = nc.dram_tensor("xt", [128, TS, FC, 512], BF16, kind="ExternalInput").ap()
    wq_d = nc.dram_tensor("wq", [128, FC, 512], BF16, kind="ExternalInput").ap()
    wk_d = nc.dram_tensor("wk", [128, FC, 512], BF16, kind="ExternalInput").ap()
    wv_d = nc.dram_tensor("wv", [128, FC, 512], BF16, kind="ExternalInput").ap()
    wo_d = nc.dram_tensor("wout", [128, 4, D], BF16, kind="ExternalInput").ap()
    out_d = nc.dram_tensor("out", [T, D], FP32, kind="ExternalOutput").ap()
    with tile.TileContext(nc) as tc:
        with ExitStack() as ctx:
            _body(ctx, nc, tc, x_d, wq_d, wk_d, wv_d, wo_d, out_d)
    nc.compile()
    return nc


_nc = None


def _get_nc():
    global _nc
    if _nc is None:
        _nc = build()
    return _nc


def make_in_maps(x, Wqkv, Wout):
    bf = ml_dtypes.bfloat16
    in_maps = []
    for c in range(8):
        b, g = divmod(c, 2)
        cs = slice(g * 512, (g + 1) * 512)
        xt = np.ascontiguousarray(x[b].T)                      # [D, T]
        xp = xt.reshape(FC, 128, TS, 512).transpose(1, 2, 0, 3)  # [p,ts,f,tw]

        def packw(w):   # [D, 512] -> [p, f, c]
            return np.ascontiguousarray(
                w.reshape(FC, 128, 512).transpose(1, 0, 2)).astype(bf)

        wo = Wout[cs, :].reshape(4, 128, D).transpose(1, 0, 2)   # [p, c, d]
        in_maps.append({
            "xt": np.ascontiguousarray(xp).astype(bf),
            "wq": packw(Wqkv[:, 0 * D:1 * D][:, cs]),
            "wk": packw(Wqkv[:, 1 * D:2 * D][:, cs]),
            "wv": packw(Wqkv[:, 2 * D:3 * D][:, cs]),
            "wout": np.ascontiguousarray(wo).astype(bf),
        })
    return in_maps


def kernel(x, Wqkv, Wout, _trace=False):
    nc = _get_nc()
    x = np.asarray(x, dtype=np.float32)
    Wqkv = np.asarray(Wqkv, dtype=np.float32)
    Wout = np.asarray(Wout, dtype=np.float32)
    in_maps = make_in_maps(x, Wqkv, Wout)
    kwargs = {}
    if _trace:
        kwargs["trace"] = True
    res = run_bass_kernel_spmd(nc, in_maps, core_ids=list(range(8)), **kwargs)
    outs = [res.results[c]["out"] for c in range(8)]
    out = np.stack([outs[2 * b] + outs[2 * b + 1] for b in range(4)])
    if _trace:
        kernel.last_result = res
    return out


# revision 16
# speedup vs baseline: 1.0317x; 1.0077x over previous
"""Multi-head attention (B=4, T=2048, D=1024, H=16) on 8 TRN2 NeuronCores.

Sharding: core c -> (batch b = c//2, head-group g = c%2 of 8 heads).
Each core computes the qkv projection for its batch restricted to its 8
heads, full attention for those heads, and a partial output projection
(ctx_local @ Wout[rows of its heads]).  Host sums the two partials per batch.

v2 schedule, built so the PE engine (the bottleneck at ~315us of inherent
matmul work per core) never stalls:

  - all inputs host-packed so every DMA is per-partition contiguous
    (8-32KB descriptors), issued in priority order (wq, wk, x0, x1, wv,
    x2, x3, wo); x stays resident in SBUF (loaded once, 4MB).
  - "ramp" block: the (pair0, qq0) attention kc-loop interleaves the
    remaining pair-0 qk projection spans and the v projection, so exp
    starts ~12us in instead of ~43us.
  - AV matmuls deferred two k-chunks behind the S matmuls so the ACT
    exp latency is fully hidden by other PE work.
  - the next pair's qk projection is spread evenly over all remaining
    slots of the current pair (48 or 64 slots); the output projection of
    query quarter qq-1 is spread over hc3's qq slots as before.
"""

import numpy as np
import ml_dtypes
from contextlib import ExitStack

import concourse.bass as bass
import concourse.bacc as bacc
import concourse.tile as tile
from concourse import mybir
from concourse.bass_utils import run_bass_kernel_spmd

FP32 = mybir.dt.float32
BF16 = mybir.dt.bfloat16
EXP = mybir.ActivationFunctionType.Exp

D = 1024
T = 2048
HPC = 8          # heads per core
FC = 8           # feature chunks of 128 (projection contraction)
TS = 4           # token spans of 512
KC = 16          # k chunks of 128
QQ = 4           # query quarters of 512


def _norm(nc, rpool, ctx_sb, ctxp, hh, hc, qsl):
    """ctx_sb[hb:hb+64, hc, qsl] = ctxp[0:64] / ctxp[64] (sumexp row)."""
    hb = (hh % 2) * 64
    rtmp = rpool.tile([1, 512], FP32, tag="rtmp")
    nc.vector.tensor_copy(out=rtmp[:], in_=ctxp[64:65, :])
    rt = rpool.tile([1, 512], FP32, tag="rt")
    nc.vector.reciprocal_approx_fast(out=rt[:], in_=rtmp[:])
    rb = rpool.tile([64, 512], FP32, tag="rb")
    nc.gpsimd.partition_broadcast(rb[:], rt[0:1, :], channels=64)
    nc.vector.tensor_mul(ctx_sb[hb:hb + 64, hc, qsl], ctxp[0:64, :], rb[:])


def _attention(nc, ppool, spsum, cpsum, rpool, qT, kT, v_sb, ctx_sb, hc, qq,
               extra=None, post=None, post2=None):
    """Both heads of pair hc for query quarter qq.

    ``extra`` (called once per k-chunk) interleaves other PE work (the
    pair-0/v projections, the next pair's qk-projection, the output
    projection) into the ACT-bound attention stream.  ``post`` runs before
    the two trailing AV emissions (used by the ramp block for vproj(15))."""
    qsl = slice(qq * 512, (qq + 1) * 512)
    P2 = ppool.tile([128, KC, 2, 512], BF16, tag="P2")
    ctxA = cpsum.tile([65, 512], FP32, tag="ctx")
    ctxB = cpsum.tile([65, 512], FP32, tag="ctx")

    def emit_av(kc):
        for i, ctxp in ((0, ctxA), (1, ctxB)):
            nc.tensor.matmul(
                ctxp[:],
                lhsT=v_sb[:, kc, 2 * hc + i, :],
                rhs=P2[:, kc, i, :],
                start=(kc == 0), stop=(kc == KC - 1))

    for kc in range(KC):
        sps = spsum.tile([128, 2, 512], FP32, tag="S")
        for i in range(2):          # head A on rows 0-63, head B on 64-127
            b0 = i * 64
            nc.tensor.matmul(
                sps[:, i, :],
                lhsT=kT[b0:b0 + 64, hc, kc * 128:(kc + 1) * 128],
                rhs=qT[b0:b0 + 64, hc, qsl],
                start=True, stop=True)
        nc.scalar.activation(
            out=P2[:, kc, :, :], in_=sps[:, :, :], func=EXP, scale=0.125)
        if extra is not None:
            extra(kc)
        # software pipeline: AV deferred three chunks, both to hide the ACT
        # exp latency and so the next block's first AV (which reuses this
        # block's cpsum buffer) lands after the ~4us norm chain completes
        if kc >= 3:
            emit_av(kc - 3)
    if post is not None:
        post()
    emit_av(KC - 3)
    emit_av(KC - 2)
    emit_av(KC - 1)
    _norm(nc, rpool, ctx_sb, ctxA, 2 * hc, hc, qsl)
    _norm(nc, rpool, ctx_sb, ctxB, 2 * hc + 1, hc, qsl)
    if post2 is not None:
        post2()


def _body(ctx, nc, tc, x_d, wq_d, wk_d, wv_d, wo_d, out_d):
    persist = ctx.enter_context(tc.tile_pool(name="persist", bufs=1))
    x_sb = persist.tile([128, TS, FC, 512], BF16, tag="x")
    qT = persist.tile([128, 4, T], BF16, tag="qT")
    kT = persist.tile([128, 4, T], BF16, tag="kT")
    v_sb = persist.tile([128, KC, HPC, 65], BF16, tag="v")
    ctx_sb = persist.tile([128, 4, T], BF16, tag="ctx")
    wq_sb = persist.tile([128, FC, 512], BF16, tag="wq")
    wk_sb = persist.tile([128, FC, 512], BF16, tag="wk")
    wv_sb = persist.tile([128, FC, 512], BF16, tag="wv")
    wo_sb = persist.tile([128, 4, D], BF16, tag="wo")

    nc.vector.memset(v_sb[:, :, :, 64:65], 1.0)

    # DMA priority order: first qk projection (span 0) needs wq+wk+x0;
    # ramp-block extras then consume x1/wv/x2/x3; wo only at hc3.
    nc.sync.dma_start(out=x_sb[:, 0], in_=x_d[:, 0])
    nc.sync.dma_start(out=wq_sb[:], in_=wq_d[:])
    nc.sync.dma_start(out=wk_sb[:], in_=wk_d[:])
    nc.sync.dma_start(out=x_sb[:, 1], in_=x_d[:, 1])
    nc.sync.dma_start(out=wv_sb[:], in_=wv_d[:])
    nc.sync.dma_start(out=x_sb[:, 2], in_=x_d[:, 2])
    nc.sync.dma_start(out=x_sb[:, 3], in_=x_d[:, 3])
    nc.sync.dma_start(out=wo_sb[:], in_=wo_d[:])

    ps = ctx.enter_context(tc.tile_pool(name="proj", bufs=2, space="PSUM"))

    def make_qk_steps(hc):
        """64 generator steps: one fc-accumulation matmul per step of pair
        hc's qk projection (4 spans x {q,k} x 8 fc), reading resident x."""
        st = {"p": None}

        def step(s):
            unit, fc = divmod(s, FC)
            ts, qk = divmod(unit, 2)
            w_sb, dst = ((wq_sb, qT), (wk_sb, kT))[qk]
            if fc == 0:
                st["p"] = ps.tile([128, 512], FP32, tag="proj", name="qkp")
            nc.tensor.matmul(
                st["p"][:],
                lhsT=w_sb[:, fc, hc * 128:(hc + 1) * 128],
                rhs=x_sb[:, ts, fc, :],
                start=(fc == 0), stop=(fc == FC - 1))
            if fc == FC - 1:
                nc.vector.tensor_copy(
                    out=dst[:, hc, ts * 512:(ts + 1) * 512], in_=st["p"][:])
        return step

    def vproj(kc):
        ts, q4 = divmod(kc, 4)
        psv = ps.tile([128, 512], FP32, tag="proj", name="vp")
        for fc in range(FC):
            nc.tensor.matmul(
                psv[:],
                lhsT=x_sb[:, ts, fc, q4 * 128:(q4 + 1) * 128],
                rhs=wv_sb[:, fc, :],
                start=(fc == 0), stop=(fc == FC - 1))
        nc.vector.tensor_copy(
            out=v_sb[:, kc, :, 0:64],
            in_=psv[:].rearrange("p (h d) -> p h d", h=HPC))

    # pair-0 span-0 q and k projections interleaved per-fc so the first S
    # matmul can start ~0.4us (not ~3.4us) after the x0/wq/wk DMAs land
    qk0 = make_qk_steps(0)
    p_q = ps.tile([128, 512], FP32, tag="proj", name="qkp0q")
    p_k = ps.tile([128, 512], FP32, tag="proj", name="qkp0k")
    for fc in range(FC):
        nc.tensor.matmul(
            p_q[:], lhsT=wq_sb[:, fc, 0:128], rhs=x_sb[:, 0, fc, :],
            start=(fc == 0), stop=(fc == FC - 1))
        nc.tensor.matmul(
            p_k[:], lhsT=wk_sb[:, fc, 0:128], rhs=x_sb[:, 0, fc, :],
            start=(fc == 0), stop=(fc == FC - 1))
    nc.vector.tensor_copy(out=qT[:, 0, 0:512], in_=p_q[:])
    nc.vector.tensor_copy(out=kT[:, 0, 0:512], in_=p_k[:])

    with tc.tile_pool(name="P", bufs=1) as ppool, \
         tc.tile_pool(name="spsum", bufs=2, space="PSUM") as spsum, \
         tc.tile_pool(name="cpsum", bufs=2, space="PSUM") as cpsum, \
         tc.tile_pool(name="rpool", bufs=2) as rpool, \
         tc.tile_pool(name="osb", bufs=4) as osb:

        def ramp_extra(kc):
            # k projections of spans 1-3 (qq0 needs all of kT but only
            # span 0 of qT) over slots 0-11, q of span 1 over slots 12-15;
            # vproj one slot behind the kc index (wv DMA arrives later)
            if kc < 12:
                unit = 2 * (kc // 4) + 3          # k units: ts 1,2,3
                for j in range(2):
                    qk0(unit * FC + 2 * (kc % 4) + j)
            else:
                for j in range(2):                # q unit of span 1
                    qk0(2 * FC + 2 * (kc - 12) + j)
            if kc >= 1:
                vproj(kc - 1)

        def make_op_steps(qq_prev):
            """16 steps emitting the output projection of qq_prev's tokens
            (4 token chunks x 2 column halves x accumulate 4 cc)."""
            st = {"po": None, "ot": None}

            def step(s):
                unit, half = divmod(s, 2)
                tcg = qq_prev * 4 + unit // 2
                j2 = unit % 2
                if half == 0:
                    if j2 == 0:
                        st["ot"] = osb.tile([128, D], FP32, tag="ot", name="ot")
                    st["po"] = ps.tile([128, 512], FP32, tag="proj", name="po")
                    ccs = (0, 1)
                else:
                    ccs = (2, 3)
                for cc in ccs:
                    nc.tensor.matmul(
                        st["po"][:],
                        lhsT=ctx_sb[:, cc, tcg * 128:(tcg + 1) * 128],
                        rhs=wo_sb[:, cc, j2 * 512:(j2 + 1) * 512],
                        start=(cc == 0), stop=(cc == 3))
                if half == 1:
                    nc.vector.tensor_copy(
                        out=st["ot"][:, j2 * 512:(j2 + 1) * 512], in_=st["po"][:])
                    if j2 == 1:
                        nc.sync.dma_start(
                            out=out_d[tcg * 128:(tcg + 1) * 128, :],
                            in_=st["ot"][:])
            return step

        # tail helper: cc0-2 of qq3's output projection depend only on
        # pairs 0-2's ctx, so they can overlap the final norm chain
        pos = {}

        def emit_cc012(u):
            tcg, j2 = 12 + u // 2, u % 2
            po = ps.tile([128, 512], FP32, tag="proj", name="po")
            for cc in range(3):
                nc.tensor.matmul(
                    po[:],
                    lhsT=ctx_sb[:, cc, tcg * 128:(tcg + 1) * 128],
                    rhs=wo_sb[:, cc, j2 * 512:(j2 + 1) * 512],
                    start=(cc == 0), stop=False)
            pos[u] = po

        for hc in range(4):
            nxt = make_qk_steps(hc + 1) if hc < 3 else None
            for qq in range(QQ):
                post = post2 = None
                if hc == 0 and qq == 0:
                    extra = ramp_extra
                    post = lambda: vproj(15)
                elif hc == 0 and qq >= 1:
                    # 64 steps of pair-1 proj over qq1-3, ending 4 slots
                    # early so hc1's first S isn't waiting on the copy;
                    # plus pair-0's q proj of span qq+1 in the first 4 slots
                    def extra(kc, qq=qq, nxt=nxt):
                        i = (qq - 1) * 16 + kc
                        if i < 44:
                            for s in range((i * 64) // 44, ((i + 1) * 64) // 44):
                                nxt(s)
                        if qq < 3 and kc < 4:
                            unit = 2 * (qq + 1)   # q units of spans 2, 3
                            for j in range(2):
                                qk0(unit * FC + 2 * kc + j)
                elif hc in (1, 2):
                    # 64 steps of the next pair's proj, ending 4 slots early
                    def extra(kc, qq=qq, nxt=nxt):
                        i = qq * 16 + kc
                        if i < 60:
                            for s in range((i * 64) // 60, ((i + 1) * 64) // 60):
                                nxt(s)
                elif hc == 3 and qq >= 1:
                    op_step = make_op_steps(qq - 1)

                    # start four slots in: the first cc3 op matmul reads
                    # ctx_sb written by the previous block's norm chain
                    # (~4us of DVE+gpsimd latency past the last AV); the
                    # last 4 steps run after this block's norms, filling
                    # the PE while the next norm chain drains
                    def extra(kc, op_step=op_step):
                        if kc >= 4:
                            op_step(kc - 4)
                    def post2(op_step=op_step):
                        for s in (12, 13, 14, 15):
                            op_step(s)
                else:
                    extra = None
                _attention(nc, ppool, spsum, cpsum, rpool,
                           qT, kT, v_sb, ctx_sb, hc, qq, extra=extra,
                           post=post, post2=post2)
        # tail: cc3 finishers for qq3's output projection
        emit_cc012(0)
        emit_cc012(1)
        ot = None
        for u in range(8):
            tcg, j2 = 12 + u // 2, u % 2
            if u >= 2:
                emit_cc012(u)
            po = pos.pop(u)
            nc.tensor.matmul(
                po[:],
                lhsT=ctx_sb[:, 3, tcg * 128:(tcg + 1) * 128],
                rhs=wo_sb[:, 3, j2 * 512:(j2 + 1) * 512],
                start=False, stop=True)
            if j2 == 0:
                ot = osb.tile([128, D], FP32, tag="ot", name="ot")
            nc.vector.tensor_copy(out=ot[:, j2 * 512:(j2 + 1) * 512], in_=po[:])
            if j2 == 1:
                nc.sync.dma_start(
                    out=out_d[tcg * 128:(tcg + 1) * 128, :], in_=ot[:])


def build():
    nc = bacc.Bacc("TRN2", target_bir_lowering=False, debug=False, num_devices=8)
    x_d = nc.dram_tensor("xt", [128, TS, FC, 512], BF16, kind="ExternalInput").ap()
    wq_d = nc.dram_tensor("wq", [128, FC, 512], BF16, kind="ExternalInput").ap()
    wk_d = nc.dram_tensor("wk", [128, FC, 512], BF16, kind="ExternalInput").ap()
    wv_d = nc.dram_tensor("wv", [128, FC, 512], BF16, kind="ExternalInput").ap()
    wo_d = nc.dram_tensor("wout", [128, 4, D], BF16, kind="ExternalInput").ap()
    out_d = nc.dram_tensor("out", [T, D], FP32, kind="ExternalOutput").ap()
    with tile.TileContext(nc) as tc:
        with ExitStack() as ctx:
            _body(ctx, nc, tc, x_d, wq_d, wk_d, wv_d, wo_d, out_d)
    nc.compile()
    return nc


_nc = None


def _get_nc():
    global _nc
    if _nc is None:
        _nc = build()
    return _nc


def make_in_maps(x, Wqkv, Wout):
    bf = ml_dtypes.bfloat16
    in_maps = []
    for c in range(8):
        b, g = divmod(c, 2)
        cs = slice(g * 512, (g + 1) * 512)
        xt = np.ascontiguousarray(x[b].T)                      # [D, T]
        xp = xt.reshape(FC, 128, TS, 512).transpose(1, 2, 0, 3)  # [p,ts,f,tw]

        def packw(w):   # [D, 512] -> [p, f, c]
            return np.ascontiguousarray(
                w.reshape(FC, 128, 512).transpose(1, 0, 2)).astype(bf)

        wo = Wout[cs, :].reshape(4, 128, D).transpose(1, 0, 2)   # [p, c, d]
        in_maps.append({
            "xt": np.ascontiguousarray(xp).astype(bf),
            "wq": packw(Wqkv[:, 0 * D:1 * D][:, cs]),
            "wk": packw(Wqkv[:, 1 * D:2 * D][:, cs]),
            "wv": packw(Wqkv[:, 2 * D:3 * D][:, cs]),
            "wout": np.ascontiguousarray(wo).astype(bf),
        })
    return in_maps


def kernel(x, Wqkv, Wout, _trace=False):
    nc = _get_nc()
    x = np.asarray(x, dtype=np.float32)
    Wqkv = np.asarray(Wqkv, dtype=np.float32)
    Wout = np.asarray(Wout, dtype=np.float32)
    in_maps = make_in_maps(x, Wqkv, Wout)
    kwargs = {}
    if _trace:
        kwargs["trace"] = True
    res = run_bass_kernel_spmd(nc, in_maps, core_ids=list(range(8)), **kwargs)
    outs = [res.results[c]["out"] for c in range(8)]
    out = np.stack([outs[2 * b] + outs[2 * b + 1] for b in range(4)])
    if _trace:
        kernel.last_result = res
    return out


# revision 17
# speedup vs baseline: 1.0351x; 1.0032x over previous
"""Multi-head attention (B=4, T=2048, D=1024, H=16) on 8 TRN2 NeuronCores.

Sharding: core c -> (batch b = c//2, head-group g = c%2 of 8 heads).
Each core computes the qkv projection for its batch restricted to its 8
heads, full attention for those heads, and a partial output projection
(ctx_local @ Wout[rows of its heads]).  Host sums the two partials per batch.

v2 schedule, built so the PE engine (the bottleneck at ~315us of inherent
matmul work per core) never stalls:

  - all inputs host-packed so every DMA is per-partition contiguous
    (8-32KB descriptors), issued in priority order (wq, wk, x0, x1, wv,
    x2, x3, wo); x stays resident in SBUF (loaded once, 4MB).
  - "ramp" block: the (pair0, qq0) attention kc-loop interleaves the
    remaining pair-0 qk projection spans and the v projection, so exp
    starts ~12us in instead of ~43us.
  - AV matmuls deferred two k-chunks behind the S matmuls so the ACT
    exp latency is fully hidden by other PE work.
  - the next pair's qk projection is spread evenly over all remaining
    slots of the current pair (48 or 64 slots); the output projection of
    query quarter qq-1 is spread over hc3's qq slots as before.
"""

import numpy as np
import ml_dtypes
from contextlib import ExitStack

import concourse.bass as bass
import concourse.bacc as bacc
import concourse.tile as tile
from concourse import mybir
from concourse.bass_utils import run_bass_kernel_spmd

FP32 = mybir.dt.float32
BF16 = mybir.dt.bfloat16
EXP = mybir.ActivationFunctionType.Exp

D = 1024
T = 2048
HPC = 8          # heads per core
FC = 8           # feature chunks of 128 (projection contraction)
TS = 4           # token spans of 512
KC = 16          # k chunks of 128
QQ = 4           # query quarters of 512


def _norm(nc, rpool, ctx_sb, ctxp, hh, hc, qsl):
    """ctx_sb[hb:hb+64, hc, qsl] = ctxp[0:64] / ctxp[64] (sumexp row)."""
    hb = (hh % 2) * 64
    rtmp = rpool.tile([1, 512], FP32, tag="rtmp")
    nc.vector.tensor_copy(out=rtmp[:], in_=ctxp[64:65, :])
    rt = rpool.tile([1, 512], FP32, tag="rt")
    nc.vector.reciprocal_approx_fast(out=rt[:], in_=rtmp[:])
    rb = rpool.tile([64, 512], FP32, tag="rb")
    nc.gpsimd.partition_broadcast(rb[:], rt[0:1, :], channels=64)
    nc.vector.tensor_mul(ctx_sb[hb:hb + 64, hc, qsl], ctxp[0:64, :], rb[:])


def _attention(nc, ppool, spsum, cpsum, rpool, qT, kT, v_sb, ctx_sb, hc, qq,
               extra=None, post=None, post2=None):
    """Both heads of pair hc for query quarter qq.

    ``extra`` (called once per k-chunk) interleaves other PE work (the
    pair-0/v projections, the next pair's qk-projection, the output
    projection) into the ACT-bound attention stream.  ``post`` runs before
    the two trailing AV emissions (used by the ramp block for vproj(15))."""
    qsl = slice(qq * 512, (qq + 1) * 512)
    P2 = ppool.tile([128, KC, 2, 512], BF16, tag="P2")
    ctxA = cpsum.tile([65, 512], FP32, tag="ctx")
    ctxB = cpsum.tile([65, 512], FP32, tag="ctx")

    def emit_av(kc):
        for i, ctxp in ((0, ctxA), (1, ctxB)):
            nc.tensor.matmul(
                ctxp[:],
                lhsT=v_sb[:, kc, 2 * hc + i, :],
                rhs=P2[:, kc, i, :],
                start=(kc == 0), stop=(kc == KC - 1))

    for kc in range(KC):
        sps = spsum.tile([128, 2, 512], FP32, tag="S")
        for i in range(2):          # head A on rows 0-63, head B on 64-127
            b0 = i * 64
            nc.tensor.matmul(
                sps[:, i, :],
                lhsT=kT[b0:b0 + 64, hc, kc * 128:(kc + 1) * 128],
                rhs=qT[b0:b0 + 64, hc, qsl],
                start=True, stop=True)
        nc.scalar.activation(
            out=P2[:, kc, :, :], in_=sps[:, :, :], func=EXP, scale=0.125)
        if extra is not None:
            extra(kc)
        # software pipeline: AV deferred four chunks, both to hide the ACT
        # exp latency and so the next block's first AV (which reuses this
        # block's cpsum buffer) lands after the ~4.5us norm chain completes
        if kc >= 4:
            emit_av(kc - 4)
    if post is not None:
        post()
    emit_av(KC - 4)
    emit_av(KC - 3)
    emit_av(KC - 2)
    emit_av(KC - 1)
    _norm(nc, rpool, ctx_sb, ctxA, 2 * hc, hc, qsl)
    _norm(nc, rpool, ctx_sb, ctxB, 2 * hc + 1, hc, qsl)
    if post2 is not None:
        post2()


def _body(ctx, nc, tc, x_d, wq_d, wk_d, wv_d, wo_d, out_d):
    persist = ctx.enter_context(tc.tile_pool(name="persist", bufs=1))
    x_sb = persist.tile([128, TS, FC, 512], BF16, tag="x")
    qT = persist.tile([128, 4, T], BF16, tag="qT")
    kT = persist.tile([128, 4, T], BF16, tag="kT")
    v_sb = persist.tile([128, KC, HPC, 65], BF16, tag="v")
    ctx_sb = persist.tile([128, 4, T], BF16, tag="ctx")
    wq_sb = persist.tile([128, FC, 512], BF16, tag="wq")
    wk_sb = persist.tile([128, FC, 512], BF16, tag="wk")
    wv_sb = persist.tile([128, FC, 512], BF16, tag="wv")
    wo_sb = persist.tile([128, 4, D], BF16, tag="wo")

    nc.vector.memset(v_sb[:, :, :, 64:65], 1.0)

    # DMA priority order: first qk projection (span 0) needs wq+wk+x0;
    # ramp-block extras then consume x1/wv/x2/x3; wo only at hc3.
    nc.sync.dma_start(out=x_sb[:, 0], in_=x_d[:, 0])
    nc.sync.dma_start(out=wq_sb[:], in_=wq_d[:])
    nc.sync.dma_start(out=wk_sb[:], in_=wk_d[:])
    nc.sync.dma_start(out=x_sb[:, 1], in_=x_d[:, 1])
    nc.sync.dma_start(out=wv_sb[:], in_=wv_d[:])
    nc.sync.dma_start(out=x_sb[:, 2], in_=x_d[:, 2])
    nc.sync.dma_start(out=x_sb[:, 3], in_=x_d[:, 3])
    nc.sync.dma_start(out=wo_sb[:], in_=wo_d[:])

    ps = ctx.enter_context(tc.tile_pool(name="proj", bufs=2, space="PSUM"))

    def make_qk_steps(hc):
        """64 generator steps: one fc-accumulation matmul per step of pair
        hc's qk projection (4 spans x {q,k} x 8 fc), reading resident x."""
        st = {"p": None}

        def step(s):
            unit, fc = divmod(s, FC)
            ts, qk = divmod(unit, 2)
            w_sb, dst = ((wq_sb, qT), (wk_sb, kT))[qk]
            if fc == 0:
                st["p"] = ps.tile([128, 512], FP32, tag="proj", name="qkp")
            nc.tensor.matmul(
                st["p"][:],
                lhsT=w_sb[:, fc, hc * 128:(hc + 1) * 128],
                rhs=x_sb[:, ts, fc, :],
                start=(fc == 0), stop=(fc == FC - 1))
            if fc == FC - 1:
                nc.vector.tensor_copy(
                    out=dst[:, hc, ts * 512:(ts + 1) * 512], in_=st["p"][:])
        return step

    def vproj(kc):
        ts, q4 = divmod(kc, 4)
        psv = ps.tile([128, 512], FP32, tag="proj", name="vp")
        for fc in range(FC):
            nc.tensor.matmul(
                psv[:],
                lhsT=x_sb[:, ts, fc, q4 * 128:(q4 + 1) * 128],
                rhs=wv_sb[:, fc, :],
                start=(fc == 0), stop=(fc == FC - 1))
        nc.vector.tensor_copy(
            out=v_sb[:, kc, :, 0:64],
            in_=psv[:].rearrange("p (h d) -> p h d", h=HPC))

    # pair-0 span-0 q and k projections interleaved per-fc so the first S
    # matmul can start ~0.4us (not ~3.4us) after the x0/wq/wk DMAs land
    qk0 = make_qk_steps(0)
    p_q = ps.tile([128, 512], FP32, tag="proj", name="qkp0q")
    p_k = ps.tile([128, 512], FP32, tag="proj", name="qkp0k")
    for fc in range(FC):
        nc.tensor.matmul(
            p_q[:], lhsT=wq_sb[:, fc, 0:128], rhs=x_sb[:, 0, fc, :],
            start=(fc == 0), stop=(fc == FC - 1))
        nc.tensor.matmul(
            p_k[:], lhsT=wk_sb[:, fc, 0:128], rhs=x_sb[:, 0, fc, :],
            start=(fc == 0), stop=(fc == FC - 1))
    nc.vector.tensor_copy(out=qT[:, 0, 0:512], in_=p_q[:])
    nc.vector.tensor_copy(out=kT[:, 0, 0:512], in_=p_k[:])

    with tc.tile_pool(name="P", bufs=1) as ppool, \
         tc.tile_pool(name="spsum", bufs=2, space="PSUM") as spsum, \
         tc.tile_pool(name="cpsum", bufs=2, space="PSUM") as cpsum, \
         tc.tile_pool(name="rpool", bufs=2) as rpool, \
         tc.tile_pool(name="osb", bufs=4) as osb:

        def ramp_extra(kc):
            # k projections of spans 1-3 (qq0 needs all of kT but only
            # span 0 of qT) over slots 0-11, q of span 1 over slots 12-15;
            # vproj one slot behind the kc index (wv DMA arrives later)
            if kc < 12:
                unit = 2 * (kc // 4) + 3          # k units: ts 1,2,3
                for j in range(2):
                    qk0(unit * FC + 2 * (kc % 4) + j)
            else:
                for j in range(2):                # q unit of span 1
                    qk0(2 * FC + 2 * (kc - 12) + j)
            if kc >= 1:
                vproj(kc - 1)

        def make_op_steps(qq_prev):
            """16 steps emitting the output projection of qq_prev's tokens
            (4 token chunks x 2 column halves x accumulate 4 cc)."""
            st = {"po": None, "ot": None}

            def step(s):
                unit, half = divmod(s, 2)
                tcg = qq_prev * 4 + unit // 2
                j2 = unit % 2
                if half == 0:
                    if j2 == 0:
                        st["ot"] = osb.tile([128, D], FP32, tag="ot", name="ot")
                    st["po"] = ps.tile([128, 512], FP32, tag="proj", name="po")
                    ccs = (0, 1)
                else:
                    ccs = (2, 3)
                for cc in ccs:
                    nc.tensor.matmul(
                        st["po"][:],
                        lhsT=ctx_sb[:, cc, tcg * 128:(tcg + 1) * 128],
                        rhs=wo_sb[:, cc, j2 * 512:(j2 + 1) * 512],
                        start=(cc == 0), stop=(cc == 3))
                if half == 1:
                    nc.vector.tensor_copy(
                        out=st["ot"][:, j2 * 512:(j2 + 1) * 512], in_=st["po"][:])
                    if j2 == 1:
                        nc.sync.dma_start(
                            out=out_d[tcg * 128:(tcg + 1) * 128, :],
                            in_=st["ot"][:])
            return step

        # tail helper: cc0-2 of qq3's output projection depend only on
        # pairs 0-2's ctx, so they can overlap the final norm chain
        pos = {}

        def emit_cc012(u):
            tcg, j2 = 12 + u // 2, u % 2
            po = ps.tile([128, 512], FP32, tag="proj", name="po")
            for cc in range(3):
                nc.tensor.matmul(
                    po[:],
                    lhsT=ctx_sb[:, cc, tcg * 128:(tcg + 1) * 128],
                    rhs=wo_sb[:, cc, j2 * 512:(j2 + 1) * 512],
                    start=(cc == 0), stop=False)
            pos[u] = po

        for hc in range(4):
            nxt = make_qk_steps(hc + 1) if hc < 3 else None
            for qq in range(QQ):
                post = post2 = None
                if hc == 0 and qq == 0:
                    extra = ramp_extra
                    post = lambda: vproj(15)
                elif hc == 0 and qq >= 1:
                    # 64 steps of pair-1 proj over qq1-3, ending 4 slots
                    # early so hc1's first S isn't waiting on the copy;
                    # plus pair-0's q proj of span qq+1 in the first 4 slots
                    def extra(kc, qq=qq, nxt=nxt):
                        i = (qq - 1) * 16 + kc
                        if i < 44:
                            for s in range((i * 64) // 44, ((i + 1) * 64) // 44):
                                nxt(s)
                        if qq < 3 and kc < 4:
                            unit = 2 * (qq + 1)   # q units of spans 2, 3
                            for j in range(2):
                                qk0(unit * FC + 2 * kc + j)
                elif hc in (1, 2):
                    # 64 steps of the next pair's proj, ending 4 slots early
                    def extra(kc, qq=qq, nxt=nxt):
                        i = qq * 16 + kc
                        if i < 60:
                            for s in range((i * 64) // 60, ((i + 1) * 64) // 60):
                                nxt(s)
                elif hc == 3 and qq >= 1:
                    op_step = make_op_steps(qq - 1)

                    # start four slots in: the first cc3 op matmul reads
                    # ctx_sb written by the previous block's norm chain
                    # (~4us of DVE+gpsimd latency past the last AV); the
                    # last 4 steps run after this block's norms, filling
                    # the PE while the next norm chain drains
                    def extra(kc, op_step=op_step):
                        if kc >= 4:
                            op_step(kc - 4)
                    def post2(op_step=op_step):
                        for s in (12, 13, 14, 15):
                            op_step(s)
                else:
                    extra = None
                _attention(nc, ppool, spsum, cpsum, rpool,
                           qT, kT, v_sb, ctx_sb, hc, qq, extra=extra,
                           post=post, post2=post2)
        # tail: cc3 finishers for qq3's output projection
        emit_cc012(0)
        emit_cc012(1)
        ot = None
        for u in range(8):
            tcg, j2 = 12 + u // 2, u % 2
            if u >= 2:
                emit_cc012(u)
            po = pos.pop(u)
            nc.tensor.matmul(
                po[:],
                lhsT=ctx_sb[:, 3, tcg * 128:(tcg + 1) * 128],
                rhs=wo_sb[:, 3, j2 * 512:(j2 + 1) * 512],
                start=False, stop=True)
            if j2 == 0:
                ot = osb.tile([128, D], FP32, tag="ot", name="ot")
            nc.vector.tensor_copy(out=ot[:, j2 * 512:(j2 + 1) * 512], in_=po[:])
            if j2 == 1:
                nc.sync.dma_start(
                    out=out_d[tcg * 128:(tcg + 1) * 128, :], in_=ot[:])


def build():
    nc = bacc.Bacc("TRN2", target_bir_lowering=False, debug=False, num_devices=8)
    x_d = nc.dram_tensor("xt", [128, TS, FC, 512], BF16, kind="ExternalInput").ap()
    wq_d = nc.dram_tensor("wq", [128, FC, 512], BF16, kind="ExternalInput").ap()
    wk_d = nc.dram_tensor("wk", [128, FC, 512], BF16, kind="ExternalInput").ap()
    wv_d = nc.dram_tensor("wv", [128, FC, 512], BF16, kind="ExternalInput").ap()
    wo_d = nc.dram_tensor("wout", [128, 4, D], BF16, kind="ExternalInput").ap()
    out_d = nc.dram_tensor("out", [T, D], FP32, kind="ExternalOutput").ap()
    with tile.TileContext(nc) as tc:
        with ExitStack() as ctx:
            _body(ctx, nc, tc, x_d, wq_d, wk_d, wv_d, wo_d, out_d)
    nc.compile()
    return nc


_nc = None


def _get_nc():
    global _nc
    if _nc is None:
        _nc = build()
    return _nc


def make_in_maps(x, Wqkv, Wout):
    bf = ml_dtypes.bfloat16
    in_maps = []
    for c in range(8):
        b, g = divmod(c, 2)
        cs = slice(g * 512, (g + 1) * 512)
        xt = np.ascontiguousarray(x[b].T)                      # [D, T]
        xp = xt.reshape(FC, 128, TS, 512).transpose(1, 2, 0, 3)  # [p,ts,f,tw]

        def packw(w):   # [D, 512] -> [p, f, c]
            return np.ascontiguousarray(
                w.reshape(FC, 128, 512).transpose(1, 0, 2)).astype(bf)

        wo = Wout[cs, :].reshape(4, 128, D).transpose(1, 0, 2)   # [p, c, d]
        in_maps.append({
            "xt": np.ascontiguousarray(xp).astype(bf),
            "wq": packw(Wqkv[:, 0 * D:1 * D][:, cs]),
            "wk": packw(Wqkv[:, 1 * D:2 * D][:, cs]),
            "wv": packw(Wqkv[:, 2 * D:3 * D][:, cs]),
            "wout": np.ascontiguousarray(wo).astype(bf),
        })
    return in_maps


def kernel(x, Wqkv, Wout, _trace=False):
    nc = _get_nc()
    x = np.asarray(x, dtype=np.float32)
    Wqkv = np.asarray(Wqkv, dtype=np.float32)
    Wout = np.asarray(Wout, dtype=np.float32)
    in_maps = make_in_maps(x, Wqkv, Wout)
    kwargs = {}
    if _trace:
        kwargs["trace"] = True
    res = run_bass_kernel_spmd(nc, in_maps, core_ids=list(range(8)), **kwargs)
    outs = [res.results[c]["out"] for c in range(8)]
    out = np.stack([outs[2 * b] + outs[2 * b + 1] for b in range(4)])
    if _trace:
        kernel.last_result = res
    return out


# revision 20
# speedup vs baseline: 1.0394x; 1.0042x over previous
"""Multi-head attention (B=4, T=2048, D=1024, H=16) on 8 TRN2 NeuronCores.

Sharding: core c -> (batch b = c//2, head-group g = c%2 of 8 heads).
Each core computes the qkv projection for its batch restricted to its 8
heads, full attention for those heads, and a partial output projection
(ctx_local @ Wout[rows of its heads]).  Host sums the two partials per batch.

Schedule notes — the PE engine is the bottleneck (786432 charged matmul
columns = ~340us/core at the measured ~2.29GHz effective clock, running at
~100% streaming efficiency), so everything is organized so PE never stalls:

  - all inputs host-packed so every DMA is per-partition contiguous
    (8-32KB descriptors), issued in priority order (x0, wq, wk, x1, wv,
    x2, x3, wo); x stays resident in SBUF (loaded once, 4MB).
  - "ramp" block: the (pair0, qq0) attention kc-loop interleaves pair-0's
    k projections for spans 1-3, the q projection of span 1, and the v
    projection, so exp starts ~18us in instead of ~43us.
  - AV matmuls deferred four k-chunks behind the S matmuls: hides the ACT
    exp latency and gives the previous block's norm chain (whose copies
    are front-loaded in _norm_pair) time to release the cpsum buffers.
  - the next pair's qk projection is spread over the current pair's slots,
    ending a few slots early so the next block's S never waits the copy;
    the output projection of quarter qq-1 is spread over hc3's qq slots
    starting 4 slots in (its cc3 reads the freshly-normed ctx), with the
    last 4 steps emitted after the norms; qq3's cc0-2 output projection
    partials overlap the final norm chain, cc3 finishers in the tail.
"""

import numpy as np
import ml_dtypes
from contextlib import ExitStack

import concourse.bass as bass
import concourse.bacc as bacc
import concourse.tile as tile
from concourse import mybir
from concourse.bass_utils import run_bass_kernel_spmd

FP32 = mybir.dt.float32
BF16 = mybir.dt.bfloat16
EXP = mybir.ActivationFunctionType.Exp

D = 1024
T = 2048
HPC = 8          # heads per core
FC = 8           # feature chunks of 128 (projection contraction)
TS = 4           # token spans of 512
KC = 16          # k chunks of 128
QQ = 4           # query quarters of 512


def _norm_pair(nc, rpool, ctx_sb, ctxA, ctxB, hc, qsl):
    """ctx_sb[:, hc, qsl] = ctx{A,B}[0:64] / ctx{A,B}[64] (sumexp rows).

    All four PSUM-reading copies are front-loaded so both cpsum buffers
    free ~2.5us after the last AV instead of after the full serialized
    recip/broadcast/mul chains (~5-6us), unblocking the next block's
    first AV matmuls."""
    rts, scs = [], []
    for ctxp, tg in ((ctxA, "A"), (ctxB, "B")):
        rtmp = rpool.tile([1, 512], FP32, tag="rtmp" + tg)
        nc.vector.tensor_copy(out=rtmp[:], in_=ctxp[64:65, :])
        sc = rpool.tile([64, 512], FP32, tag="sc" + tg)
        nc.vector.tensor_copy(out=sc[:], in_=ctxp[0:64, :])
        rts.append(rtmp)
        scs.append(sc)
    for i, hb in enumerate((0, 64)):
        rt = rpool.tile([1, 512], FP32, tag="rt")
        nc.vector.reciprocal_approx_fast(out=rt[:], in_=rts[i][:])
        rb = rpool.tile([64, 512], FP32, tag="rb")
        nc.gpsimd.partition_broadcast(rb[:], rt[0:1, :], channels=64)
        nc.vector.tensor_mul(ctx_sb[hb:hb + 64, hc, qsl], scs[i][:], rb[:])


def _attention(nc, ppool, spsum, cpsum, rpool, qT, kT, v_sb, ctx_sb, hc, qq,
               extra=None, post=None, post2=None):
    """Both heads of pair hc for query quarter qq.

    ``extra`` (called once per k-chunk) interleaves other PE work (the
    pair-0/v projections, the next pair's qk-projection, the output
    projection) into the ACT-bound attention stream.  ``post`` runs before
    the two trailing AV emissions (used by the ramp block for vproj(15))."""
    qsl = slice(qq * 512, (qq + 1) * 512)
    P2 = ppool.tile([128, KC, 2, 512], BF16, tag="P2")
    ctxA = cpsum.tile([65, 512], FP32, tag="ctx")
    ctxB = cpsum.tile([65, 512], FP32, tag="ctx")

    def emit_av(kc):
        for i, ctxp in ((0, ctxA), (1, ctxB)):
            nc.tensor.matmul(
                ctxp[:],
                lhsT=v_sb[:, kc, 2 * hc + i, :],
                rhs=P2[:, kc, i, :],
                start=(kc == 0), stop=(kc == KC - 1))

    for kc in range(KC):
        sps = spsum.tile([128, 2, 512], FP32, tag="S")
        for i in range(2):          # head A on rows 0-63, head B on 64-127
            b0 = i * 64
            nc.tensor.matmul(
                sps[:, i, :],
                lhsT=kT[b0:b0 + 64, hc, kc * 128:(kc + 1) * 128],
                rhs=qT[b0:b0 + 64, hc, qsl],
                start=True, stop=True)
        nc.scalar.activation(
            out=P2[:, kc, :, :], in_=sps[:, :, :], func=EXP, scale=0.125)
        if extra is not None:
            extra(kc)
        # software pipeline: AV deferred four chunks, both to hide the ACT
        # exp latency and so the next block's first AV (which reuses this
        # block's cpsum buffer) lands after the ~4.5us norm chain completes
        if kc >= 4:
            emit_av(kc - 4)
    if post is not None:
        post()
    emit_av(KC - 4)
    emit_av(KC - 3)
    emit_av(KC - 2)
    emit_av(KC - 1)
    _norm_pair(nc, rpool, ctx_sb, ctxA, ctxB, hc, qsl)
    if post2 is not None:
        post2()


def _body(ctx, nc, tc, x_d, wq_d, wk_d, wv_d, wo_d, out_d):
    persist = ctx.enter_context(tc.tile_pool(name="persist", bufs=1))
    x_sb = persist.tile([128, TS, FC, 512], BF16, tag="x")
    qT = persist.tile([128, 4, T], BF16, tag="qT")
    kT = persist.tile([128, 4, T], BF16, tag="kT")
    v_sb = persist.tile([128, KC, HPC, 65], BF16, tag="v")
    ctx_sb = persist.tile([128, 4, T], BF16, tag="ctx")
    wq_sb = persist.tile([128, FC, 512], BF16, tag="wq")
    wk_sb = persist.tile([128, FC, 512], BF16, tag="wk")
    wv_sb = persist.tile([128, FC, 512], BF16, tag="wv")
    wo_sb = persist.tile([128, 4, D], BF16, tag="wo")

    nc.vector.memset(v_sb[:, :, :, 64:65], 1.0)

    # DMA priority order: first qk projection (span 0) needs wq+wk+x0;
    # ramp-block extras then consume x1/wv/x2/x3; wo only at hc3.
    nc.sync.dma_start(out=x_sb[:, 0], in_=x_d[:, 0])
    nc.sync.dma_start(out=wq_sb[:], in_=wq_d[:])
    nc.sync.dma_start(out=wk_sb[:], in_=wk_d[:])
    nc.sync.dma_start(out=x_sb[:, 1], in_=x_d[:, 1])
    nc.sync.dma_start(out=wv_sb[:], in_=wv_d[:])
    nc.sync.dma_start(out=x_sb[:, 2], in_=x_d[:, 2])
    nc.sync.dma_start(out=x_sb[:, 3], in_=x_d[:, 3])
    nc.sync.dma_start(out=wo_sb[:], in_=wo_d[:])

    ps = ctx.enter_context(tc.tile_pool(name="proj", bufs=2, space="PSUM"))

    def make_qk_steps(hc):
        """64 generator steps: one fc-accumulation matmul per step of pair
        hc's qk projection (4 spans x {q,k} x 8 fc), reading resident x."""
        st = {"p": None}

        def step(s):
            unit, fc = divmod(s, FC)
            ts, qk = divmod(unit, 2)
            w_sb, dst = ((wq_sb, qT), (wk_sb, kT))[qk]
            if fc == 0:
                st["p"] = ps.tile([128, 512], FP32, tag="proj", name="qkp")
            nc.tensor.matmul(
                st["p"][:],
                lhsT=w_sb[:, fc, hc * 128:(hc + 1) * 128],
                rhs=x_sb[:, ts, fc, :],
                start=(fc == 0), stop=(fc == FC - 1))
            if fc == FC - 1:
                nc.vector.tensor_copy(
                    out=dst[:, hc, ts * 512:(ts + 1) * 512], in_=st["p"][:])
        return step

    def vproj(kc):
        ts, q4 = divmod(kc, 4)
        psv = ps.tile([128, 512], FP32, tag="proj", name="vp")
        for fc in range(FC):
            nc.tensor.matmul(
                psv[:],
                lhsT=x_sb[:, ts, fc, q4 * 128:(q4 + 1) * 128],
                rhs=wv_sb[:, fc, :],
                start=(fc == 0), stop=(fc == FC - 1))
        nc.vector.tensor_copy(
            out=v_sb[:, kc, :, 0:64],
            in_=psv[:].rearrange("p (h d) -> p h d", h=HPC))

    # pair-0 span-0 q and k projections interleaved per-fc so the first S
    # matmul can start ~0.4us (not ~3.4us) after the x0/wq/wk DMAs land
    qk0 = make_qk_steps(0)
    p_q = ps.tile([128, 512], FP32, tag="proj", name="qkp0q")
    p_k = ps.tile([128, 512], FP32, tag="proj", name="qkp0k")
    for fc in range(FC):
        nc.tensor.matmul(
            p_q[:], lhsT=wq_sb[:, fc, 0:128], rhs=x_sb[:, 0, fc, :],
            start=(fc == 0), stop=(fc == FC - 1))
        nc.tensor.matmul(
            p_k[:], lhsT=wk_sb[:, fc, 0:128], rhs=x_sb[:, 0, fc, :],
            start=(fc == 0), stop=(fc == FC - 1))
    nc.vector.tensor_copy(out=qT[:, 0, 0:512], in_=p_q[:])
    nc.vector.tensor_copy(out=kT[:, 0, 0:512], in_=p_k[:])

    with tc.tile_pool(name="P", bufs=1) as ppool, \
         tc.tile_pool(name="spsum", bufs=2, space="PSUM") as spsum, \
         tc.tile_pool(name="cpsum", bufs=2, space="PSUM") as cpsum, \
         tc.tile_pool(name="rpool", bufs=2) as rpool, \
         tc.tile_pool(name="osb", bufs=4) as osb:

        def ramp_extra(kc):
            # k projections of spans 1-3 (qq0 needs all of kT but only
            # span 0 of qT) over slots 0-11, q of span 1 over slots 12-15;
            # vproj one slot behind the kc index (wv DMA arrives later)
            if kc < 12:
                unit = 2 * (kc // 4) + 3          # k units: ts 1,2,3
                for j in range(2):
                    qk0(unit * FC + 2 * (kc % 4) + j)
            else:
                for j in range(2):                # q unit of span 1
                    qk0(2 * FC + 2 * (kc - 12) + j)
            if kc >= 1:
                vproj(kc - 1)

        def make_op_steps(qq_prev):
            """16 steps emitting the output projection of qq_prev's tokens
            (4 token chunks x 2 column halves x accumulate 4 cc)."""
            st = {"po": None, "ot": None}

            def step(s):
                unit, half = divmod(s, 2)
                tcg = qq_prev * 4 + unit // 2
                j2 = unit % 2
                if half == 0:
                    if j2 == 0:
                        st["ot"] = osb.tile([128, D], FP32, tag="ot", name="ot")
                    st["po"] = ps.tile([128, 512], FP32, tag="proj", name="po")
                    ccs = (0, 1)
                else:
                    ccs = (2, 3)
                for cc in ccs:
                    nc.tensor.matmul(
                        st["po"][:],
                        lhsT=ctx_sb[:, cc, tcg * 128:(tcg + 1) * 128],
                        rhs=wo_sb[:, cc, j2 * 512:(j2 + 1) * 512],
                        start=(cc == 0), stop=(cc == 3))
                if half == 1:
                    nc.vector.tensor_copy(
                        out=st["ot"][:, j2 * 512:(j2 + 1) * 512], in_=st["po"][:])
                    if j2 == 1:
                        nc.sync.dma_start(
                            out=out_d[tcg * 128:(tcg + 1) * 128, :],
                            in_=st["ot"][:])
            return step

        # tail helper: cc0-2 of qq3's output projection depend only on
        # pairs 0-2's ctx, so they can overlap the final norm chain
        pos = {}

        def emit_cc012(u):
            tcg, j2 = 12 + u // 2, u % 2
            po = ps.tile([128, 512], FP32, tag="proj", name="po")
            for cc in range(3):
                nc.tensor.matmul(
                    po[:],
                    lhsT=ctx_sb[:, cc, tcg * 128:(tcg + 1) * 128],
                    rhs=wo_sb[:, cc, j2 * 512:(j2 + 1) * 512],
                    start=(cc == 0), stop=False)
            pos[u] = po

        for hc in range(4):
            nxt = make_qk_steps(hc + 1) if hc < 3 else None
            for qq in range(QQ):
                post = post2 = None
                if hc == 0 and qq == 0:
                    extra = ramp_extra
                    post = lambda: vproj(15)
                elif hc == 0 and qq >= 1:
                    # 64 steps of pair-1 proj over qq1-3, ending 4 slots
                    # early so hc1's first S isn't waiting on the copy;
                    # plus pair-0's q proj of span qq+1 in the first 4 slots
                    def extra(kc, qq=qq, nxt=nxt):
                        i = (qq - 1) * 16 + kc
                        if i < 44:
                            for s in range((i * 64) // 44, ((i + 1) * 64) // 44):
                                nxt(s)
                        if qq < 3 and kc < 4:
                            unit = 2 * (qq + 1)   # q units of spans 2, 3
                            for j in range(2):
                                qk0(unit * FC + 2 * kc + j)
                elif hc in (1, 2):
                    # 64 steps of the next pair's proj, ending 4 slots early
                    def extra(kc, qq=qq, nxt=nxt):
                        i = qq * 16 + kc
                        if i < 60:
                            for s in range((i * 64) // 60, ((i + 1) * 64) // 60):
                                nxt(s)
                elif hc == 3 and qq >= 1:
                    op_step = make_op_steps(qq - 1)

                    # start four slots in: the first cc3 op matmul reads
                    # ctx_sb written by the previous block's norm chain
                    # (~4us of DVE+gpsimd latency past the last AV); the
                    # last 4 steps run after this block's norms, filling
                    # the PE while the next norm chain drains
                    def extra(kc, op_step=op_step):
                        if kc >= 4:
                            op_step(kc - 4)
                    def post2(op_step=op_step):
                        for s in (12, 13, 14, 15):
                            op_step(s)
                else:
                    extra = None
                _attention(nc, ppool, spsum, cpsum, rpool,
                           qT, kT, v_sb, ctx_sb, hc, qq, extra=extra,
                           post=post, post2=post2)
        # tail: cc3 finishers for qq3's output projection
        emit_cc012(0)
        emit_cc012(1)
        ot = None
        for u in range(8):
            tcg, j2 = 12 + u // 2, u % 2
            if u >= 2:
                emit_cc012(u)
            po = pos.pop(u)
            nc.tensor.matmul(
                po[:],
                lhsT=ctx_sb[:, 3, tcg * 128:(tcg + 1) * 128],
                rhs=wo_sb[:, 3, j2 * 512:(j2 + 1) * 512],
                start=False, stop=True)
            if j2 == 0:
                ot = osb.tile([128, D], FP32, tag="ot", name="ot")
            nc.vector.tensor_copy(out=ot[:, j2 * 512:(j2 + 1) * 512], in_=po[:])
            if j2 == 1:
                nc.sync.dma_start(
                    out=out_d[tcg * 128:(tcg + 1) * 128, :], in_=ot[:])


def build():
    nc = bacc.Bacc("TRN2", target_bir_lowering=False, debug=False, num_devices=8)
    x_d = nc.dram_tensor("xt", [128, TS, FC, 512], BF16, kind="ExternalInput").ap()
    wq_d = nc.dram_tensor("wq", [128, FC, 512], BF16, kind="ExternalInput").ap()
    wk_d = nc.dram_tensor("wk", [128, FC, 512], BF16, kind="ExternalInput").ap()
    wv_d = nc.dram_tensor("wv", [128, FC, 512], BF16, kind="ExternalInput").ap()
    wo_d = nc.dram_tensor("wout", [128, 4, D], BF16, kind="ExternalInput").ap()
    out_d = nc.dram_tensor("out", [T, D], FP32, kind="ExternalOutput").ap()
    with tile.TileContext(nc) as tc:
        with ExitStack() as ctx:
            _body(ctx, nc, tc, x_d, wq_d, wk_d, wv_d, wo_d, out_d)
    nc.compile()
    return nc


_nc = None


def _get_nc():
    global _nc
    if _nc is None:
        _nc = build()
    return _nc


def make_in_maps(x, Wqkv, Wout):
    bf = ml_dtypes.bfloat16
    in_maps = []
    for c in range(8):
        b, g = divmod(c, 2)
        cs = slice(g * 512, (g + 1) * 512)
        xt = np.ascontiguousarray(x[b].T)                      # [D, T]
        xp = xt.reshape(FC, 128, TS, 512).transpose(1, 2, 0, 3)  # [p,ts,f,tw]

        def packw(w):   # [D, 512] -> [p, f, c]
            return np.ascontiguousarray(
                w.reshape(FC, 128, 512).transpose(1, 0, 2)).astype(bf)

        wo = Wout[cs, :].reshape(4, 128, D).transpose(1, 0, 2)   # [p, c, d]
        in_maps.append({
            "xt": np.ascontiguousarray(xp).astype(bf),
            "wq": packw(Wqkv[:, 0 * D:1 * D][:, cs]),
            "wk": packw(Wqkv[:, 1 * D:2 * D][:, cs]),
            "wv": packw(Wqkv[:, 2 * D:3 * D][:, cs]),
            "wout": np.ascontiguousarray(wo).astype(bf),
        })
    return in_maps


def kernel(x, Wqkv, Wout, _trace=False):
    nc = _get_nc()
    x = np.asarray(x, dtype=np.float32)
    Wqkv = np.asarray(Wqkv, dtype=np.float32)
    Wout = np.asarray(Wout, dtype=np.float32)
    in_maps = make_in_maps(x, Wqkv, Wout)
    kwargs = {}
    if _trace:
        kwargs["trace"] = True
    res = run_bass_kernel_spmd(nc, in_maps, core_ids=list(range(8)), **kwargs)
    outs = [res.results[c]["out"] for c in range(8)]
    out = np.stack([outs[2 * b] + outs[2 * b + 1] for b in range(4)])
    if _trace:
        kernel.last_result = res
    return out


# revision 23
# speedup vs baseline: 1.0454x; 1.0058x over previous
"""Multi-head attention (B=4, T=2048, D=1024, H=16) on 8 TRN2 NeuronCores.

Sharding: core c -> (batch b = c//2, head-group g = c%2 of 8 heads).
Each core computes the qkv projection for its batch restricted to its 8
heads, full attention for those heads, and a partial output projection
(ctx_local @ Wout[rows of its heads]).  Host sums the two partials per batch.

Schedule notes — the PE engine is the bottleneck (786432 charged matmul
columns = ~340us/core at the measured ~2.29GHz effective clock, running at
~100% streaming efficiency), so everything is organized so PE never stalls:

  - all inputs host-packed so every DMA is per-partition contiguous
    (8-32KB descriptors), issued in priority order (x0, wq, wk, x1, wv,
    x2, x3, wo); x stays resident in SBUF (loaded once, 4MB).
  - "ramp" block: the (pair0, qq0) attention kc-loop interleaves pair-0's
    k projections for spans 1-3, the q projection of span 1, and the v
    projection, so exp starts ~18us in instead of ~43us.
  - AV matmuls deferred four k-chunks behind the S matmuls: hides the ACT
    exp latency and gives the previous block's norm chain (whose copies
    are front-loaded in _norm_pair) time to release the cpsum buffers.
  - the next pair's qk projection is spread over the current pair's slots,
    ending a few slots early so the next block's S never waits the copy;
    the output projection of quarter qq-1 is spread over hc3's qq slots
    starting 4 slots in (its cc3 reads the freshly-normed ctx), with the
    last 4 steps emitted after the norms; qq3's cc0-2 output projection
    partials overlap the final norm chain, cc3 finishers in the tail.
"""

import numpy as np
import ml_dtypes
from contextlib import ExitStack

import concourse.bass as bass
import concourse.bacc as bacc
import concourse.tile as tile
from concourse import mybir
from concourse.bass_utils import run_bass_kernel_spmd

FP32 = mybir.dt.float32
BF16 = mybir.dt.bfloat16
EXP = mybir.ActivationFunctionType.Exp

D = 1024
T = 2048
HPC = 8          # heads per core
FC = 8           # feature chunks of 128 (projection contraction)
TS = 4           # token spans of 512
KC = 16          # k chunks of 128
QQ = 4           # query quarters of 512


def _norm_pair(nc, rpool, ctx_sb, ctxA, ctxB, hc, qsl):
    """ctx_sb[:, hc, qsl] = ctx{A,B}[0:64] / ctx{A,B}[64] (sumexp rows).

    All four PSUM-reading copies are front-loaded so both cpsum buffers
    free ~2.5us after the last AV instead of after the full serialized
    recip/broadcast/mul chains (~5-6us), unblocking the next block's
    first AV matmuls."""
    rts, scs = [], []
    for ctxp, tg in ((ctxA, "A"), (ctxB, "B")):
        rtmp = rpool.tile([1, 512], FP32, tag="rtmp" + tg)
        nc.vector.tensor_copy(out=rtmp[:], in_=ctxp[64:65, :])
        sc = rpool.tile([64, 512], FP32, tag="sc" + tg)
        nc.vector.tensor_copy(out=sc[:], in_=ctxp[0:64, :])
        rts.append(rtmp)
        scs.append(sc)
    for i, hb in enumerate((0, 64)):
        rt = rpool.tile([1, 512], FP32, tag="rt")
        nc.vector.reciprocal_approx_fast(out=rt[:], in_=rts[i][:])
        rb = rpool.tile([64, 512], FP32, tag="rb")
        nc.gpsimd.partition_broadcast(rb[:], rt[0:1, :], channels=64)
        nc.vector.tensor_mul(ctx_sb[hb:hb + 64, hc, qsl], scs[i][:], rb[:])


def _emit_sp(nc, spsum, qT, kT, P2, hc, qq, kc):
    """One S-pair matmul plus its exp for (hc, qq, kc) into P2[:, kc]."""
    qsl = slice(qq * 512, (qq + 1) * 512)
    sps = spsum.tile([128, 2, 512], FP32, tag="S")
    for i in range(2):              # head A on rows 0-63, head B on 64-127
        b0 = i * 64
        nc.tensor.matmul(
            sps[:, i, :],
            lhsT=kT[b0:b0 + 64, hc, kc * 128:(kc + 1) * 128],
            rhs=qT[b0:b0 + 64, hc, qsl],
            start=True, stop=True)
    nc.scalar.activation(
        out=P2[:, kc, :, :], in_=sps[:, :, :], func=EXP, scale=0.125)


def _attention(nc, ppool, spsum, cpsum, rpool, qT, kT, v_sb, ctx_sb, hc, qq,
               extra=None, post=None, post2=None, hoisted_P2=None,
               hoist_next=None):
    """Both heads of pair hc for query quarter qq.

    ``extra`` (called once per k-chunk) interleaves other PE work (the
    pair-0/v projections, the next pair's qk-projection, the output
    projection) into the ACT-bound attention stream.  ``post`` runs before
    the trailing AV emissions (used by the ramp block for vproj(15)).
    ``hoisted_P2`` is set when this block's kc=0 S-pair/exp were already
    emitted at the end of the previous block (see ``hoist_next``, which
    emits the next block's kc=0 before our trailing AVs so the ACT queue
    and the next S-pair aren't serialized behind them)."""
    qsl = slice(qq * 512, (qq + 1) * 512)
    P2 = hoisted_P2 if hoisted_P2 is not None \
        else ppool.tile([128, KC, 2, 512], BF16, tag="P2")
    ctxA = cpsum.tile([65, 512], FP32, tag="ctx")
    ctxB = cpsum.tile([65, 512], FP32, tag="ctx")

    def emit_av(kc):
        for i, ctxp in ((0, ctxA), (1, ctxB)):
            nc.tensor.matmul(
                ctxp[:],
                lhsT=v_sb[:, kc, 2 * hc + i, :],
                rhs=P2[:, kc, i, :],
                start=(kc == 0), stop=(kc == KC - 1))

    for kc in range(KC):
        if not (kc == 0 and hoisted_P2 is not None):
            _emit_sp(nc, spsum, qT, kT, P2, hc, qq, kc)
        if extra is not None:
            extra(kc)
        # software pipeline: AV deferred four chunks, both to hide the ACT
        # exp latency and so the next block's first AV (which reuses this
        # block's cpsum buffer) lands after the norm chain's PSUM-freeing
        # copies complete
        if kc >= 4:
            emit_av(kc - 4)
    if post is not None:
        post()
    next_P2 = hoist_next() if hoist_next is not None else None
    emit_av(KC - 4)
    emit_av(KC - 3)
    emit_av(KC - 2)
    emit_av(KC - 1)
    _norm_pair(nc, rpool, ctx_sb, ctxA, ctxB, hc, qsl)
    if post2 is not None:
        post2()
    return next_P2


def _body(ctx, nc, tc, x_d, wq_d, wk_d, wv_d, wo_d, out_d):
    persist = ctx.enter_context(tc.tile_pool(name="persist", bufs=1))
    x_sb = persist.tile([128, TS, FC, 512], BF16, tag="x")
    qT = persist.tile([128, 4, T], BF16, tag="qT")
    kT = persist.tile([128, 4, T], BF16, tag="kT")
    v_sb = persist.tile([128, KC, HPC, 65], BF16, tag="v")
    ctx_sb = persist.tile([128, 4, T], BF16, tag="ctx")
    wq_sb = persist.tile([128, FC, 512], BF16, tag="wq")
    wk_sb = persist.tile([128, FC, 512], BF16, tag="wk")
    wv_sb = persist.tile([128, FC, 512], BF16, tag="wv")
    wo_sb = persist.tile([128, 4, D], BF16, tag="wo")

    nc.vector.memset(v_sb[:, :, :, 64:65], 1.0)

    # DMA priority order: first qk projection (span 0) needs wq+wk+x0;
    # ramp-block extras then consume x1/wv/x2/x3; wo only at hc3.
    nc.sync.dma_start(out=x_sb[:, 0], in_=x_d[:, 0])
    nc.sync.dma_start(out=wq_sb[:], in_=wq_d[:])
    nc.sync.dma_start(out=wk_sb[:], in_=wk_d[:])
    nc.sync.dma_start(out=x_sb[:, 1], in_=x_d[:, 1])
    nc.sync.dma_start(out=wv_sb[:], in_=wv_d[:])
    nc.sync.dma_start(out=x_sb[:, 2], in_=x_d[:, 2])
    nc.sync.dma_start(out=x_sb[:, 3], in_=x_d[:, 3])
    nc.sync.dma_start(out=wo_sb[:], in_=wo_d[:])

    ps = ctx.enter_context(tc.tile_pool(name="proj", bufs=2, space="PSUM"))

    def make_qk_steps(hc):
        """64 generator steps: one fc-accumulation matmul per step of pair
        hc's qk projection (4 spans x {q,k} x 8 fc), reading resident x."""
        st = {"p": None}

        def step(s):
            unit, fc = divmod(s, FC)
            ts, qk = divmod(unit, 2)
            w_sb, dst = ((wq_sb, qT), (wk_sb, kT))[qk]
            if fc == 0:
                st["p"] = ps.tile([128, 512], FP32, tag="proj", name="qkp")
            nc.tensor.matmul(
                st["p"][:],
                lhsT=w_sb[:, fc, hc * 128:(hc + 1) * 128],
                rhs=x_sb[:, ts, fc, :],
                start=(fc == 0), stop=(fc == FC - 1))
            if fc == FC - 1:
                nc.vector.tensor_copy(
                    out=dst[:, hc, ts * 512:(ts + 1) * 512], in_=st["p"][:])
        return step

    def vproj(kc):
        ts, q4 = divmod(kc, 4)
        psv = ps.tile([128, 512], FP32, tag="proj", name="vp")
        for fc in range(FC):
            nc.tensor.matmul(
                psv[:],
                lhsT=x_sb[:, ts, fc, q4 * 128:(q4 + 1) * 128],
                rhs=wv_sb[:, fc, :],
                start=(fc == 0), stop=(fc == FC - 1))
        nc.vector.tensor_copy(
            out=v_sb[:, kc, :, 0:64],
            in_=psv[:].rearrange("p (h d) -> p h d", h=HPC))

    # pair-0 span-0 q and k projections interleaved per-fc so the first S
    # matmul can start ~0.4us (not ~3.4us) after the x0/wq/wk DMAs land
    qk0 = make_qk_steps(0)
    p_q = ps.tile([128, 512], FP32, tag="proj", name="qkp0q")
    p_k = ps.tile([128, 512], FP32, tag="proj", name="qkp0k")
    for fc in range(FC):
        nc.tensor.matmul(
            p_q[:], lhsT=wq_sb[:, fc, 0:128], rhs=x_sb[:, 0, fc, :],
            start=(fc == 0), stop=(fc == FC - 1))
        nc.tensor.matmul(
            p_k[:], lhsT=wk_sb[:, fc, 0:128], rhs=x_sb[:, 0, fc, :],
            start=(fc == 0), stop=(fc == FC - 1))
    nc.vector.tensor_copy(out=qT[:, 0, 0:512], in_=p_q[:])
    nc.vector.tensor_copy(out=kT[:, 0, 0:512], in_=p_k[:])

    with tc.tile_pool(name="P", bufs=1) as ppool, \
         tc.tile_pool(name="spsum", bufs=2, space="PSUM") as spsum, \
         tc.tile_pool(name="cpsum", bufs=2, space="PSUM") as cpsum, \
         tc.tile_pool(name="rpool", bufs=2) as rpool, \
         tc.tile_pool(name="osb", bufs=4) as osb:

        def ramp_extra(kc):
            # k projections of spans 1-3 (qq0 needs all of kT but only
            # span 0 of qT) over slots 0-11, q of span 1 over slots 12-15;
            # vproj one slot behind the kc index (wv DMA arrives later)
            if kc < 12:
                unit = 2 * (kc // 4) + 3          # k units: ts 1,2,3
                for j in range(2):
                    qk0(unit * FC + 2 * (kc % 4) + j)
            else:
                for j in range(2):                # q unit of span 1
                    qk0(2 * FC + 2 * (kc - 12) + j)
            if kc >= 1:
                vproj(kc - 1)

        def make_op_steps(qq_prev):
            """16 steps emitting the output projection of qq_prev's tokens
            (4 token chunks x 2 column halves x accumulate 4 cc)."""
            st = {"po": None, "ot": None}

            def step(s):
                unit, half = divmod(s, 2)
                tcg = qq_prev * 4 + unit // 2
                j2 = unit % 2
                if half == 0:
                    if j2 == 0:
                        st["ot"] = osb.tile([128, D], FP32, tag="ot", name="ot")
                    st["po"] = ps.tile([128, 512], FP32, tag="proj", name="po")
                    ccs = (0, 1)
                else:
                    ccs = (2, 3)
                for cc in ccs:
                    nc.tensor.matmul(
                        st["po"][:],
                        lhsT=ctx_sb[:, cc, tcg * 128:(tcg + 1) * 128],
                        rhs=wo_sb[:, cc, j2 * 512:(j2 + 1) * 512],
                        start=(cc == 0), stop=(cc == 3))
                if half == 1:
                    nc.vector.tensor_copy(
                        out=st["ot"][:, j2 * 512:(j2 + 1) * 512], in_=st["po"][:])
                    if j2 == 1:
                        nc.sync.dma_start(
                            out=out_d[tcg * 128:(tcg + 1) * 128, :],
                            in_=st["ot"][:])
            return step

        # tail helper: cc0-2 of qq3's output projection depend only on
        # pairs 0-2's ctx, so they can overlap the final norm chain
        pos = {}

        def emit_cc012(u):
            tcg, j2 = 12 + u // 2, u % 2
            po = ps.tile([128, 512], FP32, tag="proj", name="po")
            for cc in range(3):
                nc.tensor.matmul(
                    po[:],
                    lhsT=ctx_sb[:, cc, tcg * 128:(tcg + 1) * 128],
                    rhs=wo_sb[:, cc, j2 * 512:(j2 + 1) * 512],
                    start=(cc == 0), stop=False)
            pos[u] = po

        hoisted_P2 = None
        for hc in range(4):
            nxt = make_qk_steps(hc + 1) if hc < 3 else None
            for qq in range(QQ):
                if (hc, qq) != (3, 3):
                    nhc, nqq = (hc, qq + 1) if qq < 3 else (hc + 1, 0)

                    def hoist_next(nhc=nhc, nqq=nqq):
                        P2n = ppool.tile([128, KC, 2, 512], BF16, tag="P2")
                        _emit_sp(nc, spsum, qT, kT, P2n, nhc, nqq, 0)
                        return P2n
                else:
                    hoist_next = None
                post = post2 = None
                if hc == 0 and qq == 0:
                    extra = ramp_extra
                    post = lambda: vproj(15)
                elif hc == 0 and qq >= 1:
                    # 64 steps of pair-1 proj over qq1-3, ending 4 slots
                    # early so hc1's first S isn't waiting on the copy;
                    # plus pair-0's q proj of span qq+1 in the first 4 slots
                    def extra(kc, qq=qq, nxt=nxt):
                        i = (qq - 1) * 16 + kc
                        if i < 44:
                            for s in range((i * 64) // 44, ((i + 1) * 64) // 44):
                                nxt(s)
                        if qq < 3 and kc < 4:
                            unit = 2 * (qq + 1)   # q units of spans 2, 3
                            for j in range(2):
                                qk0(unit * FC + 2 * kc + j)
                elif hc in (1, 2):
                    # 64 steps of the next pair's proj, ending 4 slots early
                    def extra(kc, qq=qq, nxt=nxt):
                        i = qq * 16 + kc
                        if i < 60:
                            for s in range((i * 64) // 60, ((i + 1) * 64) // 60):
                                nxt(s)
                elif hc == 3 and qq >= 1:
                    op_step = make_op_steps(qq - 1)

                    # start four slots in: the first cc3 op matmul reads
                    # ctx_sb written by the previous block's norm chain
                    # (~4us of DVE+gpsimd latency past the last AV); the
                    # last 4 steps run after this block's norms, filling
                    # the PE while the next norm chain drains
                    def extra(kc, op_step=op_step):
                        if kc >= 4:
                            op_step(kc - 4)
                    def post2(op_step=op_step):
                        for s in (12, 13, 14, 15):
                            op_step(s)
                else:
                    extra = None
                hoisted_P2 = _attention(nc, ppool, spsum, cpsum, rpool,
                                        qT, kT, v_sb, ctx_sb, hc, qq,
                                        extra=extra, post=post, post2=post2,
                                        hoisted_P2=hoisted_P2,
                                        hoist_next=hoist_next)
        # tail: cc3 finishers for qq3's output projection
        emit_cc012(0)
        emit_cc012(1)
        ot = None
        for u in range(8):
            tcg, j2 = 12 + u // 2, u % 2
            if u >= 2:
                emit_cc012(u)
            po = pos.pop(u)
            nc.tensor.matmul(
                po[:],
                lhsT=ctx_sb[:, 3, tcg * 128:(tcg + 1) * 128],
                rhs=wo_sb[:, 3, j2 * 512:(j2 + 1) * 512],
                start=False, stop=True)
            if j2 == 0:
                ot = osb.tile([128, D], FP32, tag="ot", name="ot")
            nc.vector.tensor_copy(out=ot[:, j2 * 512:(j2 + 1) * 512], in_=po[:])
            if j2 == 1:
                nc.sync.dma_start(
                    out=out_d[tcg * 128:(tcg + 1) * 128, :], in_=ot[:])


def build():
    nc = bacc.Bacc("TRN2", target_bir_lowering=False, debug=False, num_devices=8)
    x_d = nc.dram_tensor("xt", [128, TS, FC, 512], BF16, kind="ExternalInput").ap()
    wq_d = nc.dram_tensor("wq", [128, FC, 512], BF16, kind="ExternalInput").ap()
    wk_d = nc.dram_tensor("wk", [128, FC, 512], BF16, kind="ExternalInput").ap()
    wv_d = nc.dram_tensor("wv", [128, FC, 512], BF16, kind="ExternalInput").ap()
    wo_d = nc.dram_tensor("wout", [128, 4, D], BF16, kind="ExternalInput").ap()
    out_d = nc.dram_tensor("out", [T, D], FP32, kind="ExternalOutput").ap()
    with tile.TileContext(nc) as tc:
        with ExitStack() as ctx:
            _body(ctx, nc, tc, x_d, wq_d, wk_d, wv_d, wo_d, out_d)
    nc.compile()
    return nc


_nc = None


def _get_nc():
    global _nc
    if _nc is None:
        _nc = build()
    return _nc


def make_in_maps(x, Wqkv, Wout):
    bf = ml_dtypes.bfloat16
    in_maps = []
    for c in range(8):
        b, g = divmod(c, 2)
        cs = slice(g * 512, (g + 1) * 512)
        xt = np.ascontiguousarray(x[b].T)                      # [D, T]
        xp = xt.reshape(FC, 128, TS, 512).transpose(1, 2, 0, 3)  # [p,ts,f,tw]

        def packw(w):   # [D, 512] -> [p, f, c]
            return np.ascontiguousarray(
                w.reshape(FC, 128, 512).transpose(1, 0, 2)).astype(bf)

        wo = Wout[cs, :].reshape(4, 128, D).transpose(1, 0, 2)   # [p, c, d]
        in_maps.append({
            "xt": np.ascontiguousarray(xp).astype(bf),
            "wq": packw(Wqkv[:, 0 * D:1 * D][:, cs]),
            "wk": packw(Wqkv[:, 1 * D:2 * D][:, cs]),
            "wv": packw(Wqkv[:, 2 * D:3 * D][:, cs]),
            "wout": np.ascontiguousarray(wo).astype(bf),
        })
    return in_maps


def kernel(x, Wqkv, Wout, _trace=False):
    nc = _get_nc()
    x = np.asarray(x, dtype=np.float32)
    Wqkv = np.asarray(Wqkv, dtype=np.float32)
    Wout = np.asarray(Wout, dtype=np.float32)
    in_maps = make_in_maps(x, Wqkv, Wout)
    kwargs = {}
    if _trace:
        kwargs["trace"] = True
    res = run_bass_kernel_spmd(nc, in_maps, core_ids=list(range(8)), **kwargs)
    outs = [res.results[c]["out"] for c in range(8)]
    out = np.stack([outs[2 * b] + outs[2 * b + 1] for b in range(4)])
    if _trace:
        kernel.last_result = res
    return out
